# revision 1
# baseline (speedup 1.0000x reference)
"""Trainium2 Bass kernel for the nn_EncoderBlock problem.

Full inputs in, full output out. 8-way SPMD: cores 0-3 handle batch 0,
cores 4-7 batch 1. No cross-core communication (collectives are
pathologically slow and flaky under this container's runtime), so
block 1 is computed redundantly for the whole batch on each core (its
output x2 feeds block-2 K/V for every token), while block-2 attention
and output rows are 4-way query-sharded within the batch group.

All 8 cores run the SAME program: attention is permutation-invariant
over keys, so the host rotates each core's token order by its query
offset -- "queries 0..1023" on the device are exactly the core's own
output shard, while the key set stays complete.

Per block: LN(LN(x)) -> QKV projections -> per-head attention with
scores kept transposed [keys, queries] so softmax's exp doubles as the
PSUM->SBUF evacuation on the scalar engine (no max-subtraction needed:
|scores| < 2), P*V via a ones-augmented V (M=65) so the softmax
denominator falls out of the same matmul, normalization via a K=1
outer-product broadcast matmul, output projection with bias folded in
as a K=1 matmul, residual add. bf16 matmul operands, f32 accumulation,
f32 residual stream. Score matmuls are row-packed two heads at a time
(K=64 pairs on array rows 0-63/64-127).
"""

import sys

sys.path.insert(0, "/opt/trn_rl_repo")

import numpy as np
import ml_dtypes

import bass_rust
import concourse.bass as bass
import concourse.tile as tile
from concourse import mybir
from concourse.bass_utils import run_bass_kernel_spmd

F32 = mybir.dt.float32
F32R = mybir.dt.float32r
BF16 = mybir.dt.bfloat16
AF = mybir.ActivationFunctionType
ALU = mybir.AluOpType

P = 128
D = 384
H = 6
DK = 64
DT = D // P          # 3 D-chunks of 128
S = 4096             # full sequence per batch
NTS = S // P         # 32 token tiles of 128
NKC = S // P         # 32 key chunks of 128
OWN = 1024           # block-2 query tokens owned per core
GRP = 4              # cores per batch group
EPS = 1e-6
QT = 512             # query tile (free dim of score matmuls)
KCG = 3              # key chunks per exp group (3*512 = 1536 psum cols)
VROW = H * (DK + 1)  # 390: per-kc row of V_aug (64 data cols + ones col/head)

# ---------------------------------------------------------------------------
# walrus in this container caps sync-waits per instruction (1 for most,
# 0 for DMA-transpose). Hoist excess waits onto same-engine NoOps.
_WAIT_LIMIT_BY_TYPE = {"InstDmaTransposeAnt": 0}
_wfix_ctr = [0]


def _fix_sync_waits(nc):
    for f in nc.m.functions:
        for bb in f.blocks:
            out = []
            changed = False
            for ins in bb.instructions:
                si = ins.sync_info
                waits = list(si.on_wait) if si is not None else []
                limit = _WAIT_LIMIT_BY_TYPE.get(type(ins).__name__, 1)
                if len(waits) > limit:
                    keep, hoist = waits[:limit], waits[limit:]
                    for w in hoist:
                        _wfix_ctr[0] += 1
                        nop = mybir.InstNoOp(
                            name=f"WFIX-{_wfix_ctr[0]}", engine=ins.engine
                        )
                        nop.sync_info = bass_rust.SyncInfo(on_wait=[w], on_update=[])
                        out.append(nop)
                    ins.sync_info = bass_rust.SyncInfo(
                        on_wait=keep, on_update=list(si.on_update)
                    )
                    changed = True
                out.append(ins)
            if changed:
                bb.instructions = out


def _rows(dram_ap, row0, nrows):
    """[nrows, D] f32 rows of a [*, D] DRAM tensor as a DMA AP."""
    return bass.AP(tensor=dram_ap.tensor,
                   offset=dram_ap.offset + row0 * D,
                   ap=[[D, nrows], [1, D]])


# ---------------------------------------------------------------------------
def _emit_ln_tile(nc, pools, C, x_src_d, n, blk):
    """LN(LN(x)) for one 128-token tile -> zb (bf16). Pure DVE + tiny ACT,
    zero PSUM usage, so these interleave into attention without stalling
    the in-order PE stream."""
    work = pools["work"]

    def _ln_pass(src_ap, m_ra, m_rb, dst_ap, uid):
        mv = work.tile([P, 6 + 2], F32, tag="ln_mv", name=f"mv_{uid}")
        nc.vector.bn_stats(out=mv[:, 0:6], in_=src_ap)
        nc.vector.bn_aggr(out=mv[:, 6:8], in_=mv[:, 0:6])
        r = work.tile([P, 1], F32, tag="ln_r", name=f"r_{uid}")
        nc.scalar.activation(out=r[:, :], in_=mv[:, 7:8], func=AF.Ln,
                             scale=float(D) / float(D - 1))
        nc.scalar.activation(out=r[:, :], in_=r[:, :], func=AF.Exp,
                             scale=0.5)
        nc.vector.tensor_scalar_add(out=r[:, :], in0=r[:, :], scalar1=EPS)
        nc.vector.reciprocal(out=r[:, :], in_=r[:, :])
        t = work.tile([P, D], F32, tag="ln_t", name=f"t_{uid}")
        nc.vector.tensor_scalar(
            out=t[:, :], in0=src_ap,
            scalar1=mv[:, 6:7], scalar2=r[:, 0:1],
            op0=ALU.subtract, op1=ALU.mult)
        nc.vector.tensor_mul(out=t[:, :], in0=t[:, :], in1=m_ra[:, :])
        nc.vector.tensor_add(out=dst_ap, in0=t[:, :], in1=m_rb[:, :])

    xt = work.tile([P, D], F32, tag="x_ln", name=f"xln{blk}_{n}")
    nc.sync.dma_start(out=xt[:, :], in_=_rows(x_src_d, n * P, P))
    yt = work.tile([P, D], F32, tag="y1", name=f"y1_{blk}_{n}")
    _ln_pass(xt[:, :], C["ra_bc"], C["rb_bc"], yt[:, :], f"{blk}_{n}a")
    zb = work.tile([P, D], BF16, tag=f"zb{blk}",
                   bufs=(NTS if blk == 1 else 4),
                   name=f"zb_{blk}_{n}")
    _ln_pass(yt[:, :], C["a0_bc"], C["b0_bc"], zb[:, :], f"{blk}_{n}b")
    return zb


# ---------------------------------------------------------------------------
def _build_block(nc, pools, C, x_src_d, out_d, nq, blk,
                 premade_zb=None, after_qt=None):
    """One residual MSA block.

    x_src_d: DRAM AP [S, D] f32 -- input rows, full batch
    out_d:   DRAM AP [>=nq, D] f32 -- gets x_src[0:nq] + MSA(...)[0:nq]
    nq:      number of query rows (from token 0) to compute/output
    """
    work, psA, psB, ste_pool, otp = (pools[k] for k in
                                     ("work", "psA", "psB", "ste", "ot"))
    nqt = nq // QT

    # ---- LN(LN(x)) -> zb (inline or premade) -> zT via PE transpose ----
    zT = C["zT"]
    for n in range(NTS):
        if premade_zb is not None:
            zb = premade_zb[n]
        else:
            zb = _emit_ln_tile(nc, pools, C, x_src_d, n, blk)
        for dt_ in range(DT):
            tp = psB.tile([P, P], BF16, tag="acc", name=f"tp{blk}_{n}_{dt_}")
            nc.tensor.transpose(out=tp[:, 0:P],
                                in_=zb[:, dt_ * P:(dt_ + 1) * P],
                                identity=C["ident"][:, :])
            # evacuate on ACT: it is idle in the LN valley / boundary,
            # while DVE is the critical engine there
            nc.scalar.copy(out=zT[:, dt_, n * P:(n + 1) * P],
                           in_=tp[:, 0:P])

    # ---- projections: Qt (first nq tokens), Kt + V_aug (all S) ----
    qt_sb, kt_sb, v_aug = C["qt"], C["kt"], C["v_aug"]
    for (wT, b_col, dst, ncols) in ((C["wqT"], C["bq_col"], qt_sb, nq),
                                    (C["wkT"], C["bk_col"], kt_sb, S)):
        for dt_ in range(DT):
            for ntk in range(ncols // QT):
                ps = psB.tile([P, QT], F32, tag="acc",
                              name=f"p{blk}_{dt_}_{ntk}_{ncols}")
                for ki in range(DT):
                    nc.tensor.matmul(
                        ps[:, :],
                        lhsT=wT[:, ki, dt_ * P:(dt_ + 1) * P],
                        rhs=zT[:, ki, ntk * QT:(ntk + 1) * QT],
                        start=(ki == 0), stop=(ki == DT - 1))
                nc.vector.tensor_scalar(
                    out=dst[:, dt_, ntk * QT:(ntk + 1) * QT], in0=ps[:, :],
                    scalar1=b_col[:, dt_:dt_ + 1], scalar2=None, op0=ALU.add)
    for n in range(NTS):
        ps = psB.tile([P, QT], F32, tag="acc", name=f"v{blk}_{n}")
        for ki in range(DT):
            nc.tensor.matmul(
                ps[:, :D],
                lhsT=zT[:, ki, n * P:(n + 1) * P],
                rhs=C["wvT"][:, ki, :],
                start=(ki == 0), stop=(ki == DT - 1))
        # ones-augmented 65-stride layout; ones at j=64 persist from memset
        nc.vector.tensor_tensor(
            out=v_aug[:, n, :].rearrange(
                "p (h j) -> p h j", h=H, j=DK + 1)[:, :, 0:DK],
            in0=ps[:, :D].rearrange("p (h j) -> p h j", h=H, j=DK),
            in1=C["bv_bc"][:, :].rearrange("p (h j) -> p h j", h=H, j=DK),
            op=ALU.add)

    # ---- attention (queries 0..nq) + per-qt output projection ----
    n_groups = (NKC + KCG - 1) // KCG
    for ntk in range(nqt):
        ot = otp.tile([P, DT, QT], BF16, tag="ot", name=f"ot{blk}_{ntk}")
        for hp in range(DT):
            pv = [psB.tile([P, QT], F32, tag="acc",
                           name=f"pv{blk}_{ntk}_{hp}_{i}") for i in range(2)]
            for g in range(n_groups):
                kcs = list(range(g * KCG, min(NKC, (g + 1) * KCG)))
                w = len(kcs) * QT
                for half in range(2):   # head pair on partitions 0-63/64-127
                    lo = half * DK
                    st = psA.tile([P, KCG * QT], F32, tag="st",
                                  name=f"st{blk}_{ntk}_{hp}_{g}_{half}")
                    for j, kc in enumerate(kcs):
                        nc.tensor.matmul(
                            st[:, j * QT:(j + 1) * QT],
                            lhsT=kt_sb[lo:lo + DK, hp, kc * P:(kc + 1) * P],
                            rhs=qt_sb[lo:lo + DK, hp,
                                      ntk * QT:(ntk + 1) * QT],
                            start=True, stop=True)
                    ste = ste_pool.tile([P, KCG * QT], BF16, tag="ste",
                                        name=f"se{blk}_{ntk}_{hp}_{g}_{half}")
                    nc.scalar.activation(out=ste[:, :w], in_=st[:, :w],
                                         func=AF.Exp, scale=1.0 / 8.0)
                    h = 2 * hp + half
                    for j, kc in enumerate(kcs):
                        nc.tensor.matmul(
                            pv[half][0:DK + 1, :],
                            lhsT=v_aug[:, kc,
                                       h * (DK + 1):(h + 1) * (DK + 1)],
                            rhs=ste[:, j * QT:(j + 1) * QT],
                            start=(kc == 0), stop=(kc == NKC - 1),
                            skip_group_check=True)
            for half in range(2):
                lo = half * DK
                r_row = work.tile([1, QT], F32R, tag="r_row",
                                  name=f"rr{blk}_{ntk}_{hp}_{half}")
                with nc.allow_low_precision(
                        reason="f32r broadcast of softmax denom"):
                    nc.vector.reciprocal(
                        out=r_row[:, :], in_=pv[half][DK:DK + 1, :])
                r_bc = psA.tile([P, KCG * QT], F32, tag="st",
                                name=f"rb{blk}_{ntk}_{hp}_{half}")
                nc.tensor.matmul(
                    r_bc[0:DK, 0:QT],
                    lhsT=C["ones"][0:1, 0:DK],
                    rhs=r_row[0:1, :],
                    start=True, stop=True)
                r_sb = work.tile([DK, QT], F32, tag="r_sb",
                                 name=f"rs{blk}_{ntk}_{hp}_{half}")
                nc.vector.tensor_copy(out=r_sb[:, :], in_=r_bc[0:DK, 0:QT])
                nc.vector.tensor_tensor(
                    out=ot[lo:lo + DK, hp, :],
                    in0=pv[half][0:DK, :], in1=r_sb[:, :], op=ALU.mult)
        # output projection + bias + residual for this query tile
        for c4 in range(QT // P):
            tok = ntk * QT + c4 * P
            ps = psB.tile([P, QT], F32, tag="acc",
                          name=f"o{blk}_{ntk}_{c4}")
            for ki in range(DT):
                nc.tensor.matmul(
                    ps[:, :D],
                    lhsT=ot[:, ki, c4 * P:(c4 + 1) * P],
                    rhs=C["woT"][:, ki, :],
                    start=(ki == 0), stop=False)
            nc.tensor.matmul(
                ps[:, :D],
                lhsT=C["ones"][0:1, 0:P],
                rhs=C["bo_row"][0:1, :],
                start=False, stop=True, skip_group_check=True)
            xr = work.tile([P, D], F32, tag="x_res",
                           name=f"xr{blk}_{ntk}_{c4}")
            nc.sync.dma_start(out=xr[:, :], in_=_rows(x_src_d, tok, P))
            xo = work.tile([P, D], F32, tag="x_out",
                           name=f"xo{blk}_{ntk}_{c4}")
            nc.vector.tensor_tensor(
                out=xo[:, :], in0=ps[:, :D], in1=xr[:, :], op=ALU.add)
            nc.sync.dma_start(out=_rows(out_d, tok, P), in_=xo[:, :])
        if after_qt is not None:
            after_qt(ntk)


def _build_program():
    nc = bass.Bass("TRN2", target_bir_lowering=False, debug=False,
                   num_devices=8)

    di = {}
    di["xs"] = nc.dram_tensor("xs", [S, D], F32, kind="ExternalInput")
    for w in ("wqT", "wkT", "wvT", "woT"):
        di[w] = nc.dram_tensor(w, [D, D], BF16, kind="ExternalInput")
    di["bq_col"] = nc.dram_tensor("bq_col", [P, DT], F32, kind="ExternalInput")
    di["bk_col"] = nc.dram_tensor("bk_col", [P, DT], F32, kind="ExternalInput")
    di["bv_bc"] = nc.dram_tensor("bv_bc", [P, D], F32, kind="ExternalInput")
    di["bo_row"] = nc.dram_tensor("bo_row", [1, D], F32R, kind="ExternalInput")
    for w in ("ra0_bc", "rb0_bc", "ra1_bc", "rb1_bc", "a0_bc", "b0_bc"):
        di[w] = nc.dram_tensor(w, [P, D], F32, kind="ExternalInput")
    di["ones_in"] = nc.dram_tensor("ones_in", [1, P], F32R,
                                   kind="ExternalInput")
    di["ident_in"] = nc.dram_tensor("ident_in", [P, P], BF16,
                                    kind="ExternalInput")
    out_d = nc.dram_tensor("out", [OWN, D], F32, kind="ExternalOutput")
    x2_d = nc.dram_tensor("x2buf", [S, D], F32)   # internal

    with tile.TileContext(nc) as tc:
        with tc.tile_pool(name="const", bufs=1) as const, \
             tc.tile_pool(name="work", bufs=3) as work, \
             tc.tile_pool(name="ot", bufs=2) as otp, \
             tc.tile_pool(name="ste", bufs=6) as ste_pool, \
             tc.tile_pool(name="psA", bufs=2, space="PSUM") as psA, \
             tc.tile_pool(name="psB", bufs=2, space="PSUM") as psB:

            pools = {"work": work, "psA": psA, "psB": psB,
                     "ste": ste_pool, "ot": otp}

            C = {}
            for wname in ("wqT", "wkT", "wvT", "woT"):
                C[wname] = const.tile([P, DT, D], BF16, name=wname)
                nc.sync.dma_start(
                    out=C[wname][:, :, :],
                    in_=di[wname][:].rearrange("(d p) e -> p d e", p=P))
            for wname in ("bq_col", "bk_col", "bv_bc"):
                C[wname] = const.tile(list(di[wname].shape), F32, name=wname)
                nc.sync.dma_start(out=C[wname][:], in_=di[wname][:])
            C["bo_row"] = const.tile([1, D], F32R, name="bo_row")
            nc.sync.dma_start(out=C["bo_row"][:], in_=di["bo_row"][:])
            for wname in ("ra0_bc", "rb0_bc", "ra1_bc", "rb1_bc",
                          "a0_bc", "b0_bc"):
                C[wname] = const.tile([P, D], F32, name=wname)
                nc.sync.dma_start(out=C[wname][:, :], in_=di[wname][:])
            C["ones"] = const.tile([1, P], F32R, name="ones")
            nc.sync.dma_start(out=C["ones"][:, :], in_=di["ones_in"][:])
            C["ident"] = const.tile([P, P], BF16, name="ident")
            nc.sync.dma_start(out=C["ident"][:, :], in_=di["ident_in"][:])

            C["zT"] = const.tile([P, DT, S], BF16, name="zT")
            C["qt"] = const.tile([P, DT, S], BF16, name="qt")
            C["kt"] = const.tile([P, DT, S], BF16, name="kt")
            C["v_aug"] = const.tile([P, NTS, VROW], BF16, name="v_aug")
            nc.vector.memset(C["v_aug"][:, :, :], 1.0)

            C1 = dict(C)
            C1["ra_bc"], C1["rb_bc"] = C["ra0_bc"], C["rb0_bc"]
            C2 = dict(C)
            C2["ra_bc"], C2["rb_bc"] = C["ra1_bc"], C["rb1_bc"]

            # block-2 LN interleaves into block-1 attention: as each query
            # tile's x2 rows land, emit the pure-DVE LN chain for them.
            zb2 = [None] * NTS

            def _b2_ln(ntk):
                for n in range(ntk * 4, ntk * 4 + 4):
                    zb2[n] = _emit_ln_tile(nc, pools, C2, x2_d[:], n, 1)

            _build_block(nc, pools, C1, di["xs"][:], x2_d[:], S, 0,
                         after_qt=_b2_ln)
            _build_block(nc, pools, C2, x2_d[:], out_d[:], OWN, 1,
                         premade_zb=zb2)

    _fix_sync_waits(nc)
    return nc


_NC_CACHE = None


def _get_nc():
    global _NC_CACHE
    if _NC_CACHE is None:
        _NC_CACHE = _build_program()
    return _NC_CACHE


def _prep_inputs(x, a0, b0, ra0, rb0, ra1, rb1,
                 wq, bq, wk, bk, wv, bv, wo, bo):
    bf = ml_dtypes.bfloat16
    base = {
        "wqT": np.ascontiguousarray(np.asarray(wq, np.float32).T).astype(bf),
        "wkT": np.ascontiguousarray(np.asarray(wk, np.float32).T).astype(bf),
        "wvT": np.ascontiguousarray(np.asarray(wv, np.float32).T).astype(bf),
        "woT": np.ascontiguousarray(np.asarray(wo, np.float32).T).astype(bf),
        "bq_col": np.ascontiguousarray(
            np.asarray(bq, np.float32).reshape(DT, P).T),
        "bk_col": np.ascontiguousarray(
            np.asarray(bk, np.float32).reshape(DT, P).T),
        "bv_bc": np.ascontiguousarray(
            np.broadcast_to(np.asarray(bv, np.float32), (P, D))),
        "bo_row": np.asarray(bo, np.float32).reshape(1, D).copy(),
        "ra0_bc": np.ascontiguousarray(
            np.broadcast_to(np.asarray(ra0, np.float32), (P, D))),
        "rb0_bc": np.ascontiguousarray(
            np.broadcast_to(np.asarray(rb0, np.float32), (P, D))),
        "ra1_bc": np.ascontiguousarray(
            np.broadcast_to(np.asarray(ra1, np.float32), (P, D))),
        "rb1_bc": np.ascontiguousarray(
            np.broadcast_to(np.asarray(rb1, np.float32), (P, D))),
        "a0_bc": np.ascontiguousarray(
            np.broadcast_to(np.asarray(a0, np.float32), (P, D))),
        "b0_bc": np.ascontiguousarray(
            np.broadcast_to(np.asarray(b0, np.float32), (P, D))),
        "ones_in": np.ones((1, P), np.float32),
        "ident_in": np.eye(P, dtype=np.float32).astype(bf),
    }
    x = np.asarray(x, np.float32)
    in_maps = []
    for c in range(8):
        b, q0 = c // GRP, (c % GRP) * OWN
        m = dict(base)
        # rotate tokens so this core's output shard sits at rows 0..OWN
        m["xs"] = np.ascontiguousarray(
            np.concatenate([x[b, q0:], x[b, :q0]], axis=0))
        in_maps.append(m)
    return in_maps


def kernel(**inputs):
    nc = _get_nc()
    in_maps = _prep_inputs(**inputs)
    res = run_bass_kernel_spmd(nc, in_maps, list(range(8)))
    B = inputs["x"].shape[0]
    out = np.empty((B, S, D), np.float32)
    for c in range(8):
        b, q0 = c // GRP, (c % GRP) * OWN
        out[b, q0:q0 + OWN, :] = res.results[c]["out"]
    return out


if __name__ == "__main__":
    rng = np.random.default_rng(0)
    ins = {
        "x": rng.standard_normal((2, S, D)).astype(np.float32),
        "a0": np.ones(D, np.float32), "b0": np.zeros(D, np.float32),
        "ra0": np.ones(D, np.float32), "rb0": np.zeros(D, np.float32),
        "ra1": np.ones(D, np.float32), "rb1": np.zeros(D, np.float32),
        "wq": (rng.standard_normal((D, D)) * 0.02).astype(np.float32),
        "bq": np.zeros(D, np.float32),
        "wk": (rng.standard_normal((D, D)) * 0.02).astype(np.float32),
        "bk": np.zeros(D, np.float32),
        "wv": (rng.standard_normal((D, D)) * 0.02).astype(np.float32),
        "bv": np.zeros(D, np.float32),
        "wo": (rng.standard_normal((D, D)) * 0.02).astype(np.float32),
        "bo": np.zeros(D, np.float32),
    }
    out = kernel(**ins)
    print("kernel ran, out shape", out.shape, out.dtype)



# revision 3
# speedup vs baseline: 1.2417x; 1.2417x over previous
"""Trainium2 Bass kernel for the nn_EncoderBlock problem.

Full inputs in, full output out. 8-way SPMD: cores 0-3 handle batch 0,
cores 4-7 batch 1. Within each 4-core batch group, BOTH blocks are
query-sharded 4 ways (1024 owned rows per core). Between the blocks, the
post-LayerNorm activations zb2 = LN(LN(x2)) (bf16, the exact values the
matmuls would consume anyway) are exchanged with a DRAM AllGather over
replica groups [[0..3],[4..7]] so every core can build block-2 K/V for
the full sequence. Block-2 keys are consumed in global row order (the
gather's order); attention is permutation-invariant over keys, so this
coexists with each core's rotated local query order.

All 8 cores run the SAME program: the host rotates each core's token
order by its query offset -- "queries 0..1023" on the device are exactly
the core's own output shard, while the key set stays complete.

Per block: LN(LN(x)) -> QKV projections -> per-head attention with
scores kept transposed [keys, queries] so softmax's exp doubles as the
PSUM->SBUF evacuation on the scalar engine (no max-subtraction needed:
|scores| < 2), P*V via a ones-augmented V (M=65) so the softmax
denominator falls out of the same matmul, normalization via a K=1
outer-product broadcast matmul, output projection with bias folded in
as a K=1 matmul, residual add. bf16 matmul operands, f32 accumulation,
f32 residual stream. Score matmuls are row-packed two heads at a time
(K=64 pairs on array rows 0-63/64-127).
"""

import sys

sys.path.insert(0, "/opt/trn_rl_repo")

import numpy as np
import ml_dtypes

import bass_rust
import concourse.bass as bass
import concourse.tile as tile
from concourse import mybir
from concourse.bass_utils import run_bass_kernel_spmd

F32 = mybir.dt.float32
F32R = mybir.dt.float32r
BF16 = mybir.dt.bfloat16
AF = mybir.ActivationFunctionType
ALU = mybir.AluOpType

P = 128
D = 384
H = 6
DK = 64
DT = D // P          # 3 D-chunks of 128
S = 4096             # full sequence per batch
NTS = S // P         # 32 token tiles of 128
NKC = S // P         # 32 key chunks of 128
OWN = 1024           # query tokens owned per core (both blocks)
NOT = OWN // P       # 8 owned token tiles
GRP = 4              # cores per batch group
EPS = 1e-6
QT = 512             # query tile (free dim of score matmuls)
NQT = OWN // QT      # 2 query tiles per block
KCG = 3              # key chunks per exp group (3*512 = 1536 psum cols)
VROW = H * (DK + 1)  # 390: per-kc row of V_aug (64 data cols + ones col/head)
CC_GROUPS = [[0, 1, 2, 3], [4, 5, 6, 7]]

# ---------------------------------------------------------------------------
# walrus in this container caps sync-waits per instruction (1 for most,
# 0 for DMA-transpose). Hoist excess waits onto same-engine NoOps.
_WAIT_LIMIT_BY_TYPE = {"InstDmaTransposeAnt": 0}
_wfix_ctr = [0]


def _fix_sync_waits(nc):
    for f in nc.m.functions:
        for bb in f.blocks:
            out = []
            changed = False
            for ins in bb.instructions:
                si = ins.sync_info
                waits = list(si.on_wait) if si is not None else []
                limit = _WAIT_LIMIT_BY_TYPE.get(type(ins).__name__, 1)
                if len(waits) > limit:
                    keep, hoist = waits[:limit], waits[limit:]
                    for w in hoist:
                        _wfix_ctr[0] += 1
                        nop = mybir.InstNoOp(
                            name=f"WFIX-{_wfix_ctr[0]}", engine=ins.engine
                        )
                        nop.sync_info = bass_rust.SyncInfo(on_wait=[w], on_update=[])
                        out.append(nop)
                    ins.sync_info = bass_rust.SyncInfo(
                        on_wait=keep, on_update=list(si.on_update)
                    )
                    changed = True
                out.append(ins)
            if changed:
                bb.instructions = out


def _rows(dram_ap, row0, nrows):
    """[nrows, D] rows of a [*, D] DRAM tensor as a DMA AP."""
    return bass.AP(tensor=dram_ap.tensor,
                   offset=dram_ap.offset + row0 * D,
                   ap=[[D, nrows], [1, D]])


# ---------------------------------------------------------------------------
def _emit_ln_tile(nc, pools, C, x_src_d, n, blk):
    """LN(LN(x)) for one 128-token tile -> zb (bf16). Pure DVE + tiny ACT,
    zero PSUM usage, so these interleave into attention without stalling
    the in-order PE stream."""
    work = pools["work"]

    def _ln_pass(src_ap, m_ra, m_rb, dst_ap, uid):
        mv = work.tile([P, 6 + 2], F32, tag="ln_mv", name=f"mv_{uid}")
        nc.vector.bn_stats(out=mv[:, 0:6], in_=src_ap)
        nc.vector.bn_aggr(out=mv[:, 6:8], in_=mv[:, 0:6])
        r = work.tile([P, 1], F32, tag="ln_r", name=f"r_{uid}")
        nc.scalar.activation(out=r[:, :], in_=mv[:, 7:8], func=AF.Ln,
                             scale=float(D) / float(D - 1))
        nc.scalar.activation(out=r[:, :], in_=r[:, :], func=AF.Exp,
                             scale=0.5)
        nc.vector.tensor_scalar_add(out=r[:, :], in0=r[:, :], scalar1=EPS)
        nc.vector.reciprocal(out=r[:, :], in_=r[:, :])
        t = work.tile([P, D], F32, tag="ln_t", name=f"t_{uid}")
        nc.vector.tensor_scalar(
            out=t[:, :], in0=src_ap,
            scalar1=mv[:, 6:7], scalar2=r[:, 0:1],
            op0=ALU.subtract, op1=ALU.mult)
        nc.vector.tensor_mul(out=t[:, :], in0=t[:, :], in1=m_ra[:, :])
        nc.vector.tensor_add(out=dst_ap, in0=t[:, :], in1=m_rb[:, :])

    xt = work.tile([P, D], F32, tag="x_ln", name=f"xln{blk}_{n}")
    nc.sync.dma_start(out=xt[:, :], in_=_rows(x_src_d, n * P, P))
    yt = work.tile([P, D], F32, tag="y1", name=f"y1_{blk}_{n}")
    _ln_pass(xt[:, :], C["ra_bc"], C["rb_bc"], yt[:, :], f"{blk}_{n}a")
    zb = work.tile([P, D], BF16, tag=f"zb{blk}",
                   bufs=(NOT if blk == 1 else 4),
                   name=f"zb_{blk}_{n}")
    _ln_pass(yt[:, :], C["a0_bc"], C["b0_bc"], zb[:, :], f"{blk}_{n}b")
    return zb


def _transpose_tile(nc, pools, C, zb_ap, dst, n, uid):
    """3 PE transposes of a [128, D] bf16 tile into dst[:, dt, n*P:...]."""
    psB = pools["psB"]
    for dt_ in range(DT):
        tp = psB.tile([P, P], BF16, tag="acc", name=f"tp{uid}_{n}_{dt_}")
        nc.tensor.transpose(out=tp[:, 0:P],
                            in_=zb_ap[:, dt_ * P:(dt_ + 1) * P],
                            identity=C["ident"][:, :])
        # evacuate on ACT: it is idle in the LN valley / boundary,
        # while DVE is the critical engine there
        nc.scalar.copy(out=dst[:, dt_, n * P:(n + 1) * P],
                       in_=tp[:, 0:P])


# ---------------------------------------------------------------------------
def _build_block(nc, pools, C, x_src_d, out_d, blk,
                 zb_dram=None, premade_zb=None, after_qt=None):
    """One residual MSA block, query-sharded to OWN rows.

    x_src_d: DRAM AP [*, D] f32 -- local rows (rotated order); rows
             0..OWN are this core's queries and residual base.
    out_d:   DRAM AP [OWN, D] f32 -- gets x_src[0:OWN] + MSA(...)[0:OWN]
    zb_dram: if set (block 2), [S, D] bf16 DRAM with LN(LN(x2)) for the
             WHOLE sequence in global order -- K/V source. If None
             (block 1), LN chains run locally on x_src_d for all S rows.
    premade_zb: block 2 only -- the 8 locally-computed own zb tiles
             (rotated order), transposed into zTq for the Q projection.
    """
    work, psA, psB, ste_pool, otp = (pools[k] for k in
                                     ("work", "psA", "psB", "ste", "ot"))

    # ---- K/V source: zT [P, DT, S] ----
    zT = C["zT"]
    for n in range(NTS):
        if zb_dram is not None:
            zb = work.tile([P, D], BF16, tag="zb_g", name=f"zbg_{n}")
            nc.sync.dma_start(out=zb[:, :], in_=_rows(zb_dram, n * P, P))
        else:
            zb = _emit_ln_tile(nc, pools, C, x_src_d, n, blk)
        _transpose_tile(nc, pools, C, zb[:, :], zT, n, f"z{blk}")

    # ---- Q source: zTq [P, DT, OWN] ----
    if premade_zb is None:
        zTq = None  # block 1: zT rows 0..OWN are exactly the own queries
    else:
        zTq = C["zTq"]
        for n in range(NOT):
            _transpose_tile(nc, pools, C, premade_zb[n][:, :], zTq, n,
                            f"q{blk}")

    # ---- projections: Qt (own rows), Kt + V_aug (all S) ----
    qt_sb, kt_sb, v_aug = C["qt"], C["kt"], C["v_aug"]
    q_src = zTq if zTq is not None else zT
    for (wT, b_col, src, dst, ncols) in (
            (C["wqT"], C["bq_col"], q_src, qt_sb, OWN),
            (C["wkT"], C["bk_col"], zT, kt_sb, S)):
        for dt_ in range(DT):
            for ntk in range(ncols // QT):
                ps = psB.tile([P, QT], F32, tag="acc",
                              name=f"p{blk}_{dt_}_{ntk}_{ncols}")
                for ki in range(DT):
                    nc.tensor.matmul(
                        ps[:, :],
                        lhsT=wT[:, ki, dt_ * P:(dt_ + 1) * P],
                        rhs=src[:, ki, ntk * QT:(ntk + 1) * QT],
                        start=(ki == 0), stop=(ki == DT - 1))
                nc.vector.tensor_scalar(
                    out=dst[:, dt_, ntk * QT:(ntk + 1) * QT], in0=ps[:, :],
                    scalar1=b_col[:, dt_:dt_ + 1], scalar2=None, op0=ALU.add)
    for n in range(NTS):
        ps = psB.tile([P, QT], F32, tag="acc", name=f"v{blk}_{n}")
        for ki in range(DT):
            nc.tensor.matmul(
                ps[:, :D],
                lhsT=zT[:, ki, n * P:(n + 1) * P],
                rhs=C["wvT"][:, ki, :],
                start=(ki == 0), stop=(ki == DT - 1))
        # ones-augmented 65-stride layout; ones at j=64 persist from memset
        nc.vector.tensor_tensor(
            out=v_aug[:, n, :].rearrange(
                "p (h j) -> p h j", h=H, j=DK + 1)[:, :, 0:DK],
            in0=ps[:, :D].rearrange("p (h j) -> p h j", h=H, j=DK),
            in1=C["bv_bc"][:, :].rearrange("p (h j) -> p h j", h=H, j=DK),
            op=ALU.add)

    # ---- attention (own queries) + per-qt output projection ----
    n_groups = (NKC + KCG - 1) // KCG
    for ntk in range(NQT):
        ot = otp.tile([P, DT, QT], BF16, tag="ot", name=f"ot{blk}_{ntk}")
        for hp in range(DT):
            pv = [psB.tile([P, QT], F32, tag="acc",
                           name=f"pv{blk}_{ntk}_{hp}_{i}") for i in range(2)]
            for g in range(n_groups):
                kcs = list(range(g * KCG, min(NKC, (g + 1) * KCG)))
                w = len(kcs) * QT
                for half in range(2):   # head pair on partitions 0-63/64-127
                    lo = half * DK
                    st = psA.tile([P, KCG * QT], F32, tag="st",
                                  name=f"st{blk}_{ntk}_{hp}_{g}_{half}")
                    for j, kc in enumerate(kcs):
                        nc.tensor.matmul(
                            st[:, j * QT:(j + 1) * QT],
                            lhsT=kt_sb[lo:lo + DK, hp, kc * P:(kc + 1) * P],
                            rhs=qt_sb[lo:lo + DK, hp,
                                      ntk * QT:(ntk + 1) * QT],
                            start=True, stop=True)
                    ste = ste_pool.tile([P, KCG * QT], BF16, tag="ste",
                                        name=f"se{blk}_{ntk}_{hp}_{g}_{half}")
                    nc.scalar.activation(out=ste[:, :w], in_=st[:, :w],
                                         func=AF.Exp, scale=1.0 / 8.0)
                    h = 2 * hp + half
                    for j, kc in enumerate(kcs):
                        nc.tensor.matmul(
                            pv[half][0:DK + 1, :],
                            lhsT=v_aug[:, kc,
                                       h * (DK + 1):(h + 1) * (DK + 1)],
                            rhs=ste[:, j * QT:(j + 1) * QT],
                            start=(kc == 0), stop=(kc == NKC - 1),
                            skip_group_check=True)
            for half in range(2):
                lo = half * DK
                r_row = work.tile([1, QT], F32R, tag="r_row",
                                  name=f"rr{blk}_{ntk}_{hp}_{half}")
                with nc.allow_low_precision(
                        reason="f32r broadcast of softmax denom"):
                    nc.vector.reciprocal(
                        out=r_row[:, :], in_=pv[half][DK:DK + 1, :])
                r_bc = psA.tile([P, KCG * QT], F32, tag="st",
                                name=f"rb{blk}_{ntk}_{hp}_{half}")
                nc.tensor.matmul(
                    r_bc[0:DK, 0:QT],
                    lhsT=C["ones"][0:1, 0:DK],
                    rhs=r_row[0:1, :],
                    start=True, stop=True)
                r_sb = work.tile([DK, QT], F32, tag="r_sb",
                                 name=f"rs{blk}_{ntk}_{hp}_{half}")
                nc.vector.tensor_copy(out=r_sb[:, :], in_=r_bc[0:DK, 0:QT])
                nc.vector.tensor_tensor(
                    out=ot[lo:lo + DK, hp, :],
                    in0=pv[half][0:DK, :], in1=r_sb[:, :], op=ALU.mult)
        # output projection + bias + residual for this query tile
        for c4 in range(QT // P):
            tok = ntk * QT + c4 * P
            ps = psB.tile([P, QT], F32, tag="acc",
                          name=f"o{blk}_{ntk}_{c4}")
            for ki in range(DT):
                nc.tensor.matmul(
                    ps[:, :D],
                    lhsT=ot[:, ki, c4 * P:(c4 + 1) * P],
                    rhs=C["woT"][:, ki, :],
                    start=(ki == 0), stop=False)
            nc.tensor.matmul(
                ps[:, :D],
                lhsT=C["ones"][0:1, 0:P],
                rhs=C["bo_row"][0:1, :],
                start=False, stop=True, skip_group_check=True)
            xr = work.tile([P, D], F32, tag="x_res",
                           name=f"xr{blk}_{ntk}_{c4}")
            nc.sync.dma_start(out=xr[:, :], in_=_rows(x_src_d, tok, P))
            xo = work.tile([P, D], F32, tag="x_out",
                           name=f"xo{blk}_{ntk}_{c4}")
            nc.vector.tensor_tensor(
                out=xo[:, :], in0=ps[:, :D], in1=xr[:, :], op=ALU.add)
            nc.sync.dma_start(out=_rows(out_d, tok, P), in_=xo[:, :])
        if after_qt is not None:
            after_qt(ntk)


def _build_program():
    nc = bass.Bass("TRN2", target_bir_lowering=False, debug=False,
                   num_devices=8)

    di = {}
    di["xs"] = nc.dram_tensor("xs", [S, D], F32, kind="ExternalInput")
    for w in ("wqT", "wkT", "wvT", "woT"):
        di[w] = nc.dram_tensor(w, [D, D], BF16, kind="ExternalInput")
    di["bq_col"] = nc.dram_tensor("bq_col", [P, DT], F32, kind="ExternalInput")
    di["bk_col"] = nc.dram_tensor("bk_col", [P, DT], F32, kind="ExternalInput")
    di["bv_bc"] = nc.dram_tensor("bv_bc", [P, D], F32, kind="ExternalInput")
    di["bo_row"] = nc.dram_tensor("bo_row", [1, D], F32R, kind="ExternalInput")
    for w in ("ra0_bc", "rb0_bc", "ra1_bc", "rb1_bc", "a0_bc", "b0_bc"):
        di[w] = nc.dram_tensor(w, [P, D], F32, kind="ExternalInput")
    di["ones_in"] = nc.dram_tensor("ones_in", [1, P], F32R,
                                   kind="ExternalInput")
    di["ident_in"] = nc.dram_tensor("ident_in", [P, P], BF16,
                                    kind="ExternalInput")
    out_d = nc.dram_tensor("out", [OWN, D], F32, kind="ExternalOutput")
    x2_d = nc.dram_tensor("x2buf", [OWN, D], F32)        # internal
    gin_d = nc.dram_tensor("gin", [OWN, D], BF16)        # gather input
    gout_d = nc.dram_tensor("gout", [S, D], BF16)        # gather output

    with tile.TileContext(nc) as tc:
        with tc.tile_pool(name="const", bufs=1) as const, \
             tc.tile_pool(name="work", bufs=3) as work, \
             tc.tile_pool(name="ot", bufs=2) as otp, \
             tc.tile_pool(name="ste", bufs=6) as ste_pool, \
             tc.tile_pool(name="psA", bufs=2, space="PSUM") as psA, \
             tc.tile_pool(name="psB", bufs=2, space="PSUM") as psB:

            pools = {"work": work, "psA": psA, "psB": psB,
                     "ste": ste_pool, "ot": otp}

            C = {}
            for wname in ("wqT", "wkT", "wvT", "woT"):
                C[wname] = const.tile([P, DT, D], BF16, name=wname)
                nc.sync.dma_start(
                    out=C[wname][:, :, :],
                    in_=di[wname][:].rearrange("(d p) e -> p d e", p=P))
            for wname in ("bq_col", "bk_col", "bv_bc"):
                C[wname] = const.tile(list(di[wname].shape), F32, name=wname)
                nc.sync.dma_start(out=C[wname][:], in_=di[wname][:])
            C["bo_row"] = const.tile([1, D], F32R, name="bo_row")
            nc.sync.dma_start(out=C["bo_row"][:], in_=di["bo_row"][:])
            for wname in ("ra0_bc", "rb0_bc", "ra1_bc", "rb1_bc",
                          "a0_bc", "b0_bc"):
                C[wname] = const.tile([P, D], F32, name=wname)
                nc.sync.dma_start(out=C[wname][:, :], in_=di[wname][:])
            C["ones"] = const.tile([1, P], F32R, name="ones")
            nc.sync.dma_start(out=C["ones"][:, :], in_=di["ones_in"][:])
            C["ident"] = const.tile([P, P], BF16, name="ident")
            nc.sync.dma_start(out=C["ident"][:, :], in_=di["ident_in"][:])

            C["zT"] = const.tile([P, DT, S], BF16, name="zT")
            C["zTq"] = const.tile([P, DT, OWN], BF16, name="zTq")
            C["qt"] = const.tile([P, DT, OWN], BF16, name="qt")
            C["kt"] = const.tile([P, DT, S], BF16, name="kt")
            C["v_aug"] = const.tile([P, NTS, VROW], BF16, name="v_aug")
            nc.vector.memset(C["v_aug"][:, :, :], 1.0)

            C1 = dict(C)
            C1["ra_bc"], C1["rb_bc"] = C["ra0_bc"], C["rb0_bc"]
            C2 = dict(C)
            C2["ra_bc"], C2["rb_bc"] = C["ra1_bc"], C["rb1_bc"]

            # block-2 LN of OWN rows interleaves into block-1 attention:
            # as each query tile's x2 rows land, run the pure-DVE LN chain,
            # stage the bf16 result into the gather input, and after the
            # last tile kick the AllGather that publishes zb2 group-wide.
            zb2 = [None] * NOT

            def _b2_ln(ntk):
                for n in range(ntk * 4, ntk * 4 + 4):
                    zb2[n] = _emit_ln_tile(nc, pools, C2, x2_d[:], n, 1)
                    nc.sync.dma_start(
                        out=bass.AP(tensor=gin_d[:].tensor,
                                    offset=n * P * D,
                                    ap=[[D, P], [1, D]]),
                        in_=zb2[n][:, :])
                if ntk == NQT - 1:
                    nc.gpsimd.collective_compute(
                        "AllGather", mybir.AluOpType.bypass,
                        replica_groups=CC_GROUPS,
                        ins=[gin_d[:].opt()], outs=[gout_d[:].opt()])

            _build_block(nc, pools, C1, di["xs"][:], x2_d[:], 0,
                         after_qt=_b2_ln)
            _build_block(nc, pools, C2, x2_d[:], out_d[:], 1,
                         zb_dram=gout_d[:], premade_zb=zb2)

    _fix_sync_waits(nc)
    return nc


_NC_CACHE = None


def _get_nc():
    global _NC_CACHE
    if _NC_CACHE is None:
        _NC_CACHE = _build_program()
    return _NC_CACHE


def _prep_inputs(x, a0, b0, ra0, rb0, ra1, rb1,
                 wq, bq, wk, bk, wv, bv, wo, bo):
    bf = ml_dtypes.bfloat16
    base = {
        "wqT": np.ascontiguousarray(np.asarray(wq, np.float32).T).astype(bf),
        "wkT": np.ascontiguousarray(np.asarray(wk, np.float32).T).astype(bf),
        "wvT": np.ascontiguousarray(np.asarray(wv, np.float32).T).astype(bf),
        "woT": np.ascontiguousarray(np.asarray(wo, np.float32).T).astype(bf),
        "bq_col": np.ascontiguousarray(
            np.asarray(bq, np.float32).reshape(DT, P).T),
        "bk_col": np.ascontiguousarray(
            np.asarray(bk, np.float32).reshape(DT, P).T),
        "bv_bc": np.ascontiguousarray(
            np.broadcast_to(np.asarray(bv, np.float32), (P, D))),
        "bo_row": np.asarray(bo, np.float32).reshape(1, D).copy(),
        "ra0_bc": np.ascontiguousarray(
            np.broadcast_to(np.asarray(ra0, np.float32), (P, D))),
        "rb0_bc": np.ascontiguousarray(
            np.broadcast_to(np.asarray(rb0, np.float32), (P, D))),
        "ra1_bc": np.ascontiguousarray(
            np.broadcast_to(np.asarray(ra1, np.float32), (P, D))),
        "rb1_bc": np.ascontiguousarray(
            np.broadcast_to(np.asarray(rb1, np.float32), (P, D))),
        "a0_bc": np.ascontiguousarray(
            np.broadcast_to(np.asarray(a0, np.float32), (P, D))),
        "b0_bc": np.ascontiguousarray(
            np.broadcast_to(np.asarray(b0, np.float32), (P, D))),
        "ones_in": np.ones((1, P), np.float32),
        "ident_in": np.eye(P, dtype=np.float32).astype(bf),
    }
    x = np.asarray(x, np.float32)
    in_maps = []
    for c in range(8):
        b, q0 = c // GRP, (c % GRP) * OWN
        m = dict(base)
        # rotate tokens so this core's output shard sits at rows 0..OWN
        m["xs"] = np.ascontiguousarray(
            np.concatenate([x[b, q0:], x[b, :q0]], axis=0))
        in_maps.append(m)
    return in_maps


def kernel(**inputs):
    nc = _get_nc()
    in_maps = _prep_inputs(**inputs)
    res = run_bass_kernel_spmd(nc, in_maps, list(range(8)))
    B = inputs["x"].shape[0]
    out = np.empty((B, S, D), np.float32)
    for c in range(8):
        b, q0 = c // GRP, (c % GRP) * OWN
        out[b, q0:q0 + OWN, :] = res.results[c]["out"]
    return out


if __name__ == "__main__":
    rng = np.random.default_rng(0)
    ins = {
        "x": rng.standard_normal((2, S, D)).astype(np.float32),
        "a0": np.ones(D, np.float32), "b0": np.zeros(D, np.float32),
        "ra0": np.ones(D, np.float32), "rb0": np.zeros(D, np.float32),
        "ra1": np.ones(D, np.float32), "rb1": np.zeros(D, np.float32),
        "wq": (rng.standard_normal((D, D)) * 0.02).astype(np.float32),
        "bq": np.zeros(D, np.float32),
        "wk": (rng.standard_normal((D, D)) * 0.02).astype(np.float32),
        "bk": np.zeros(D, np.float32),
        "wv": (rng.standard_normal((D, D)) * 0.02).astype(np.float32),
        "bv": np.zeros(D, np.float32),
        "wo": (rng.standard_normal((D, D)) * 0.02).astype(np.float32),
        "bo": np.zeros(D, np.float32),
    }
    out = kernel(**ins)
    print("kernel ran, out shape", out.shape, out.dtype)


# revision 12
# speedup vs baseline: 1.9894x; 1.6021x over previous
"""Trainium2 Bass kernel for the nn_EncoderBlock problem.

Full inputs in, full output out. 8-way SPMD: cores 0-3 handle batch 0,
cores 4-7 batch 1. Within each 4-core batch group, BOTH blocks are
query-sharded 4 ways (1024 owned rows per core). Between the blocks, the
post-LayerNorm activations zb2 = LN(LN(x2)) (bf16, the exact values the
matmuls would consume anyway) are exchanged with a DRAM AllGather over
replica groups [[0..3],[4..7]] so every core can build block-2 K/V for
the full sequence. Block-2 keys are consumed in global row order (the
gather's order); attention is permutation-invariant over keys, so this
coexists with each core's rotated local query order.

All 8 cores run the SAME program: the host rotates each core's token
order by its query offset -- "queries 0..1023" on the device are exactly
the core's own output shard, while the key set stays complete.

Per block: LN(LN(x)) -> QKV projections -> per-head attention with
scores kept transposed [keys, queries] so softmax's exp doubles as the
PSUM->SBUF evacuation on the scalar engine (no max-subtraction needed:
|scores| < 2), P*V via a ones-augmented V (M=65) so the softmax
denominator falls out of the same matmul, normalization via a K=1
outer-product broadcast matmul, output projection with bias folded in
as a K=1 matmul, residual add. bf16 matmul operands, f32 accumulation,
f32 residual stream. Score matmuls are row-packed two heads at a time
(K=64 pairs on array rows 0-63/64-127).
"""

import sys

sys.path.insert(0, "/opt/trn_rl_repo")

import numpy as np
import ml_dtypes

import bass_rust
import concourse.bass as bass
import concourse.tile as tile
from concourse import mybir
from concourse.bass_utils import run_bass_kernel_spmd

F32 = mybir.dt.float32
F32R = mybir.dt.float32r
BF16 = mybir.dt.bfloat16
FP8 = mybir.dt.float8e4
AF = mybir.ActivationFunctionType
ALU = mybir.AluOpType
DR = mybir.MatmulPerfMode.DoubleRow

P = 128
D = 384
H = 6
DK = 64
DT = D // P          # 3 D-chunks of 128
S = 4096             # full sequence per batch
NTS = S // P         # 32 token tiles of 128
NKC = S // P         # 32 key chunks of 128
OWN = 1024           # query tokens owned per core (both blocks)
NOT = OWN // P       # 8 owned token tiles
GRP = 4              # cores per batch group
EPS = 1e-6
QT = 512             # query tile (free dim of score matmuls)
NQT = OWN // QT      # 2 query tiles per block
KCG = 3              # key chunks per exp group (3*512 = 1536 psum cols)
VROW = H * (DK + 1)  # 390: per-kc row of V_aug (64 data cols + ones col/head)
VROW2 = 400          # padded to a 16-byte-multiple stride for DoubleRow
CC_GROUPS = [[0, 1, 2, 3], [4, 5, 6, 7]]

# ---------------------------------------------------------------------------
# walrus in this container caps sync-waits per instruction (1 for most,
# 0 for DMA-transpose). Hoist excess waits onto same-engine NoOps.
_WAIT_LIMIT_BY_TYPE = {"InstDmaTransposeAnt": 0}
_wfix_ctr = [0]


def _fix_sync_waits(nc):
    for f in nc.m.functions:
        for bb in f.blocks:
            out = []
            changed = False
            for ins in bb.instructions:
                si = ins.sync_info
                waits = list(si.on_wait) if si is not None else []
                limit = _WAIT_LIMIT_BY_TYPE.get(type(ins).__name__, 1)
                if len(waits) > limit:
                    keep, hoist = waits[:limit], waits[limit:]
                    for w in hoist:
                        _wfix_ctr[0] += 1
                        nop = mybir.InstNoOp(
                            name=f"WFIX-{_wfix_ctr[0]}", engine=ins.engine
                        )
                        nop.sync_info = bass_rust.SyncInfo(on_wait=[w], on_update=[])
                        out.append(nop)
                    ins.sync_info = bass_rust.SyncInfo(
                        on_wait=keep, on_update=list(si.on_update)
                    )
                    changed = True
                out.append(ins)
            if changed:
                bb.instructions = out


def _rows(dram_ap, row0, nrows):
    """[nrows, D] rows of a [*, D] DRAM tensor as a DMA AP."""
    return bass.AP(tensor=dram_ap.tensor,
                   offset=dram_ap.offset + row0 * D,
                   ap=[[D, nrows], [1, D]])


# ---------------------------------------------------------------------------
def _emit_ln_tile(nc, pools, C, x_src_d, n, blk):
    """LN(LN(x)) for one 128-token tile -> zb (bf16). Pure DVE + tiny ACT,
    zero PSUM usage, so these interleave into attention without stalling
    the in-order PE stream."""
    work = pools["work"]

    def _ln_pass(src_ap, m_ra, m_rb, dst_ap, uid):
        mv = work.tile([P, 6 + 2], F32, tag="ln_mv", name=f"mv_{uid}")
        nc.vector.bn_stats(out=mv[:, 0:6], in_=src_ap)
        nc.vector.bn_aggr(out=mv[:, 6:8], in_=mv[:, 0:6])
        r = work.tile([P, 1], F32, tag="ln_r", name=f"r_{uid}")
        nc.scalar.activation(out=r[:, :], in_=mv[:, 7:8], func=AF.Ln,
                             scale=float(D) / float(D - 1))
        nc.scalar.activation(out=r[:, :], in_=r[:, :], func=AF.Exp,
                             scale=0.5)
        nc.vector.tensor_scalar_add(out=r[:, :], in0=r[:, :], scalar1=EPS)
        nc.vector.reciprocal(out=r[:, :], in_=r[:, :])
        t = work.tile([P, D], F32, tag="ln_t", name=f"t_{uid}")
        nc.vector.tensor_scalar(
            out=t[:, :], in0=src_ap,
            scalar1=mv[:, 6:7], scalar2=r[:, 0:1],
            op0=ALU.subtract, op1=ALU.mult)
        nc.vector.tensor_mul(out=t[:, :], in0=t[:, :], in1=m_ra[:, :])
        nc.vector.tensor_add(out=dst_ap, in0=t[:, :], in1=m_rb[:, :])

    xt = work.tile([P, D], F32, tag="x_ln", name=f"xln{blk}_{n}")
    nc.sync.dma_start(out=xt[:, :], in_=_rows(x_src_d, n * P, P))
    yt = work.tile([P, D], F32, tag="y1", name=f"y1_{blk}_{n}")
    _ln_pass(xt[:, :], C["ra_bc"], C["rb_bc"], yt[:, :], f"{blk}_{n}a")
    zb = work.tile([P, D], BF16, tag=f"zb{blk}",
                   bufs=(NOT if blk == 1 else 4),
                   name=f"zb_{blk}_{n}")
    _ln_pass(yt[:, :], C["a0_bc"], C["b0_bc"], zb[:, :], f"{blk}_{n}b")
    return zb


def _transpose_tile(nc, pools, C, zb_ap, dst, n, uid):
    """3 PE transposes of a [128, D] bf16 tile into dst[:, dt, n*P:...]."""
    psB = pools["psB"]
    for dt_ in range(DT):
        tp = psB.tile([P, P], BF16, tag="acc", name=f"tp{uid}_{n}_{dt_}")
        nc.tensor.transpose(out=tp[:, 0:P],
                            in_=zb_ap[:, dt_ * P:(dt_ + 1) * P],
                            identity=C["ident"][:, :])
        # evacuate on ACT: it is idle in the LN valley / boundary,
        # while DVE is the critical engine there
        nc.scalar.copy(out=dst[:, dt_, n * P:(n + 1) * P],
                       in_=tp[:, 0:P])


# ---------------------------------------------------------------------------
def _build_block(nc, pools, C, x_src_d, out_d, blk,
                 zb_dram=None, premade_zb=None, after_qt=None):
    """One residual MSA block, query-sharded to OWN rows.

    x_src_d: DRAM AP [*, D] f32 -- local rows (rotated order); rows
             0..OWN are this core's queries and residual base.
    out_d:   DRAM AP [OWN, D] f32 -- gets x_src[0:OWN] + MSA(...)[0:OWN]
    zb_dram: if set (block 2), [S, D] bf16 DRAM with LN(LN(x2)) for the
             WHOLE sequence in global order -- K/V source. If None
             (block 1), LN chains run locally on x_src_d for all S rows.
    premade_zb: block 2 only -- the 8 locally-computed own zb tiles
             (rotated order), transposed into zTq for the Q projection.
    """
    work, psA, psB, ste_pool, otp = (pools[k] for k in
                                     ("work", "psA", "psB", "ste", "ot"))

    qt_sb, kt_sb, v_aug = C["qt"], C["kt"], C["v_aug"]
    zT = C["zT"]

    def _q_proj(src):
        for dt_ in range(DT):
            for ntk in range(OWN // QT):
                ps = psB.tile([P, QT], F32, tag="acc",
                              name=f"pq{blk}_{dt_}_{ntk}")
                for ki in range(DT):
                    nc.tensor.matmul(
                        ps[:, :],
                        lhsT=C["wqT"][:, ki, dt_ * P:(dt_ + 1) * P],
                        rhs=src[:, ki, ntk * QT:(ntk + 1) * QT],
                        start=(ki == 0), stop=(ki == DT - 1))
                nc.vector.tensor_scalar(
                    out=qt_sb[:, dt_, ntk * QT:(ntk + 1) * QT], in0=ps[:, :],
                    scalar1=C["bq_col"][:, dt_:dt_ + 1], scalar2=None,
                    op0=ALU.add)

    def _kv_tiles(n0, n1):
        """zT (from LN or gather) + K-proj + V_aug for token tiles n0..n1."""
        for n in range(n0, n1):
            if zb_dram is not None:
                half = zb_dram[n // (NTS // 2)]
                zb = work.tile([P, D], BF16, tag="zb_g", name=f"zbg_{n}")
                nc.sync.dma_start(
                    out=zb[:, :],
                    in_=_rows(half, (n % (NTS // 2)) * P, P))
            else:
                zb = _emit_ln_tile(nc, pools, C, x_src_d, n, blk)
            _transpose_tile(nc, pools, C, zb[:, :], zT, n, f"z{blk}")
        for dt_ in range(DT):
            for ntk in range(n0 * P // QT, n1 * P // QT):
                ps = psB.tile([P, QT], F32, tag="acc",
                              name=f"pk{blk}_{dt_}_{ntk}")
                for ki in range(DT):
                    nc.tensor.matmul(
                        ps[:, :],
                        lhsT=C["wkT"][:, ki, dt_ * P:(dt_ + 1) * P],
                        rhs=zT[:, ki, ntk * QT:(ntk + 1) * QT],
                        start=(ki == 0), stop=(ki == DT - 1))
                nc.vector.tensor_scalar(
                    out=kt_sb[:, dt_, ntk * QT:(ntk + 1) * QT], in0=ps[:, :],
                    scalar1=C["bk_col"][:, dt_:dt_ + 1], scalar2=None,
                    op0=ALU.add)
        for n in range(n0, n1):
            ps = psB.tile([P, QT], F32, tag="acc", name=f"v{blk}_{n}")
            for ki in range(DT):
                nc.tensor.matmul(
                    ps[:, :D],
                    lhsT=zT[:, ki, n * P:(n + 1) * P],
                    rhs=C["wvT"][:, ki, :],
                    start=(ki == 0), stop=(ki == DT - 1))
            # ones-augmented layout; ones at j=DK persist from memset
            nc.vector.tensor_tensor(
                out=v_aug[:, n, 0:VROW].rearrange(
                    "p (h j) -> p h j", h=H, j=DK + 1)[:, :, 0:DK],
                in0=ps[:, :D].rearrange("p (h j) -> p h j", h=H, j=DK),
                in1=C["bv_bc"][:, :].rearrange("p (h j) -> p h j", h=H, j=DK),
                op=ALU.add)

    if premade_zb is None:
        # block 1: all LN local; zT rows 0..OWN are exactly the own queries
        _kv_tiles(0, NTS)
        _q_proj(zT)
    else:
        # block 2: emit gather-independent work first (Q from local own
        # tiles), then K/V in gather-half order so the PE stream never
        # waits on the second gather before exhausting first-half work.
        zTq = C["zTq"]
        for n in range(NOT):
            _transpose_tile(nc, pools, C, premade_zb[n][:, :], zTq, n,
                            f"q{blk}")
        _q_proj(zTq)
        _kv_tiles(0, NTS // 2)
        _kv_tiles(NTS // 2, NTS)

    # ---- attention (own queries) + per-qt output projection ----
    n_groups = (NKC + KCG - 1) // KCG
    for ntk in range(NQT):
        ot = otp.tile([P, DT, QT], BF16, tag="ot", name=f"ot{blk}_{ntk}")
        for hp in range(DT):
            pv = [psB.tile([P, QT], F32, tag="acc",
                           name=f"pv{blk}_{ntk}_{hp}_{i}") for i in range(2)]
            for g in range(n_groups):
                kcs = list(range(g * KCG, min(NKC, (g + 1) * KCG)))
                w = len(kcs) * QT
                for half in range(2):   # head pair on partitions 0-63/64-127
                    lo = half * DK
                    st = psA.tile([P, KCG * QT], F32, tag="st",
                                  name=f"st{blk}_{ntk}_{hp}_{g}_{half}")
                    for j, kc in enumerate(kcs):
                        nc.tensor.matmul(
                            st[:, j * QT:(j + 1) * QT],
                            lhsT=kt_sb[lo:lo + DK, hp, kc * P:(kc + 1) * P],
                            rhs=qt_sb[lo:lo + DK, hp,
                                      ntk * QT:(ntk + 1) * QT],
                            start=True, stop=True)
                    ste = ste_pool.tile([P, KCG * QT], FP8, tag="ste",
                                        name=f"se{blk}_{ntk}_{hp}_{g}_{half}")
                    nc.scalar.activation(out=ste[:, :w], in_=st[:, :w],
                                         func=AF.Exp, scale=1.0 / 8.0)
                    h = 2 * hp + half
                    # P*V: fp8 DoubleRow over kc pairs (2 key chunks per
                    # matmul at 0.5 cyc/row), odd remainder plain fp8.
                    j = 0
                    while j < len(kcs):
                        kc = kcs[j]
                        if j + 1 < len(kcs):
                            nc.tensor.matmul(
                                pv[half][0:DK + 1, :],
                                lhsT=v_aug[:, kc:kc + 2,
                                           h * (DK + 1):(h + 1) * (DK + 1)],
                                rhs=ste[:, j * QT:(j + 2) * QT].rearrange(
                                    "p (k q) -> p k q", k=2, q=QT),
                                start=(kc == 0), stop=(kc + 1 == NKC - 1),
                                perf_mode=DR, skip_group_check=True)
                            j += 2
                        else:
                            nc.tensor.matmul(
                                pv[half][0:DK + 1, :],
                                lhsT=v_aug[:, kc,
                                           h * (DK + 1):(h + 1) * (DK + 1)],
                                rhs=ste[:, j * QT:(j + 1) * QT],
                                start=(kc == 0), stop=(kc == NKC - 1),
                                skip_group_check=True)
                            j += 1
            for half in range(2):
                lo = half * DK
                r_row = work.tile([1, QT], F32R, tag="r_row",
                                  name=f"rr{blk}_{ntk}_{hp}_{half}")
                with nc.allow_low_precision(
                        reason="f32r broadcast of softmax denom"):
                    nc.vector.reciprocal(
                        out=r_row[:, :], in_=pv[half][DK:DK + 1, :])
                r_bc = psA.tile([P, KCG * QT], F32, tag="st",
                                name=f"rb{blk}_{ntk}_{hp}_{half}")
                nc.tensor.matmul(
                    r_bc[0:DK, 0:QT],
                    lhsT=C["ones"][0:1, 0:DK],
                    rhs=r_row[0:1, :],
                    start=True, stop=True)
                r_sb = work.tile([DK, QT], F32, tag="r_sb",
                                 name=f"rs{blk}_{ntk}_{hp}_{half}")
                nc.vector.tensor_copy(out=r_sb[:, :], in_=r_bc[0:DK, 0:QT])
                nc.vector.tensor_tensor(
                    out=ot[lo:lo + DK, hp, :],
                    in0=pv[half][0:DK, :], in1=r_sb[:, :], op=ALU.mult)
        # output projection + bias + residual for this query tile
        for c4 in range(QT // P):
            tok = ntk * QT + c4 * P
            ps = psB.tile([P, QT], F32, tag="acc",
                          name=f"o{blk}_{ntk}_{c4}")
            for ki in range(DT):
                nc.tensor.matmul(
                    ps[:, :D],
                    lhsT=ot[:, ki, c4 * P:(c4 + 1) * P],
                    rhs=C["woT"][:, ki, :],
                    start=(ki == 0), stop=False)
            nc.tensor.matmul(
                ps[:, :D],
                lhsT=C["ones"][0:1, 0:P],
                rhs=C["bo_row"][0:1, :],
                start=False, stop=True, skip_group_check=True)
            xr = work.tile([P, D], F32, tag="x_res",
                           name=f"xr{blk}_{ntk}_{c4}")
            nc.sync.dma_start(out=xr[:, :], in_=_rows(x_src_d, tok, P))
            xo = work.tile([P, D], F32, tag="x_out",
                           name=f"xo{blk}_{ntk}_{c4}")
            nc.vector.tensor_tensor(
                out=xo[:, :], in0=ps[:, :D], in1=xr[:, :], op=ALU.add)
            nc.sync.dma_start(out=_rows(out_d, tok, P), in_=xo[:, :])
        if after_qt is not None:
            after_qt(ntk)


def _build_program():
    nc = bass.Bass("TRN2", target_bir_lowering=False, debug=False,
                   num_devices=8)

    di = {}
    di["xs"] = nc.dram_tensor("xs", [S, D], F32, kind="ExternalInput")
    for w in ("wqT", "wkT", "wvT", "woT"):
        di[w] = nc.dram_tensor(w, [D, D], BF16, kind="ExternalInput")
    di["bq_col"] = nc.dram_tensor("bq_col", [P, DT], F32, kind="ExternalInput")
    di["bk_col"] = nc.dram_tensor("bk_col", [P, DT], F32, kind="ExternalInput")
    di["bv_bc"] = nc.dram_tensor("bv_bc", [P, D], F32, kind="ExternalInput")
    di["bo_row"] = nc.dram_tensor("bo_row", [1, D], F32R, kind="ExternalInput")
    for w in ("ra0_bc", "rb0_bc", "ra1_bc", "rb1_bc", "a0_bc", "b0_bc"):
        di[w] = nc.dram_tensor(w, [P, D], F32, kind="ExternalInput")
    di["ones_in"] = nc.dram_tensor("ones_in", [1, P], F32R,
                                   kind="ExternalInput")
    di["ident_in"] = nc.dram_tensor("ident_in", [P, P], BF16,
                                    kind="ExternalInput")
    out_d = nc.dram_tensor("out", [OWN, D], F32, kind="ExternalOutput")
    x2_d = nc.dram_tensor("x2buf", [OWN, D], F32)        # internal
    # split gather: half h carries each member's own rows [h*512:(h+1)*512];
    # separate tensors so first-half consumers never falsely depend on the
    # second collective.
    gin_d = [nc.dram_tensor(f"gin{h}", [OWN // 2, D], BF16) for h in range(2)]
    gout_d = [nc.dram_tensor(f"gout{h}", [S // 2, D], BF16) for h in range(2)]

    with tile.TileContext(nc) as tc:
        with tc.tile_pool(name="const", bufs=1) as const, \
             tc.tile_pool(name="work", bufs=3) as work, \
             tc.tile_pool(name="ot", bufs=2) as otp, \
             tc.tile_pool(name="ste", bufs=6) as ste_pool, \
             tc.tile_pool(name="psA", bufs=2, space="PSUM") as psA, \
             tc.tile_pool(name="psB", bufs=2, space="PSUM") as psB:

            pools = {"work": work, "psA": psA, "psB": psB,
                     "ste": ste_pool, "ot": otp}

            C = {}
            for wname in ("wqT", "wkT", "wvT", "woT"):
                C[wname] = const.tile([P, DT, D], BF16, name=wname)
                nc.sync.dma_start(
                    out=C[wname][:, :, :],
                    in_=di[wname][:].rearrange("(d p) e -> p d e", p=P))
            for wname in ("bq_col", "bk_col", "bv_bc"):
                C[wname] = const.tile(list(di[wname].shape), F32, name=wname)
                nc.sync.dma_start(out=C[wname][:], in_=di[wname][:])
            C["bo_row"] = const.tile([1, D], F32R, name="bo_row")
            nc.sync.dma_start(out=C["bo_row"][:], in_=di["bo_row"][:])
            for wname in ("ra0_bc", "rb0_bc", "ra1_bc", "rb1_bc",
                          "a0_bc", "b0_bc"):
                C[wname] = const.tile([P, D], F32, name=wname)
                nc.sync.dma_start(out=C[wname][:, :], in_=di[wname][:])
            C["ones"] = const.tile([1, P], F32R, name="ones")
            nc.sync.dma_start(out=C["ones"][:, :], in_=di["ones_in"][:])
            C["ident"] = const.tile([P, P], BF16, name="ident")
            nc.sync.dma_start(out=C["ident"][:, :], in_=di["ident_in"][:])

            C["zT"] = const.tile([P, DT, S], BF16, name="zT")
            C["zTq"] = const.tile([P, DT, OWN], BF16, name="zTq")
            C["qt"] = const.tile([P, DT, OWN], BF16, name="qt")
            C["kt"] = const.tile([P, DT, S], BF16, name="kt")
            C["v_aug"] = const.tile([P, NTS, VROW2], FP8, name="v_aug")
            nc.vector.memset(C["v_aug"][:, :, :], 1.0)

            C1 = dict(C)
            C1["ra_bc"], C1["rb_bc"] = C["ra0_bc"], C["rb0_bc"]
            C2 = dict(C)
            C2["ra_bc"], C2["rb_bc"] = C["ra1_bc"], C["rb1_bc"]

            # block-2 LN of OWN rows interleaves into block-1 attention:
            # as each query tile's x2 rows land, run the pure-DVE LN chain,
            # stage the bf16 result into that half's gather input, and kick
            # the half-AllGather that publishes it group-wide. The first
            # gather runs concurrently with block-1's second query tile.
            zb2 = [None] * NOT

            def _b2_ln(ntk):
                for n in range(ntk * 4, ntk * 4 + 4):
                    zb2[n] = _emit_ln_tile(nc, pools, C2, x2_d[:], n, 1)
                    nc.sync.dma_start(
                        out=_rows(gin_d[ntk][:], (n % 4) * P, P),
                        in_=zb2[n][:, :])
                nc.gpsimd.collective_compute(
                    "AllGather", mybir.AluOpType.bypass,
                    replica_groups=CC_GROUPS,
                    ins=[gin_d[ntk][:].opt()], outs=[gout_d[ntk][:].opt()])

            _build_block(nc, pools, C1, di["xs"][:], x2_d[:], 0,
                         after_qt=_b2_ln)
            _build_block(nc, pools, C2, x2_d[:], out_d[:], 1,
                         zb_dram=[g[:] for g in gout_d], premade_zb=zb2)

    _fix_sync_waits(nc)
    return nc


_NC_CACHE = None


def _get_nc():
    global _NC_CACHE
    if _NC_CACHE is None:
        _NC_CACHE = _build_program()
    return _NC_CACHE


def _prep_inputs(x, a0, b0, ra0, rb0, ra1, rb1,
                 wq, bq, wk, bk, wv, bv, wo, bo):
    bf = ml_dtypes.bfloat16
    base = {
        "wqT": np.ascontiguousarray(np.asarray(wq, np.float32).T).astype(bf),
        "wkT": np.ascontiguousarray(np.asarray(wk, np.float32).T).astype(bf),
        "wvT": np.ascontiguousarray(np.asarray(wv, np.float32).T).astype(bf),
        "woT": np.ascontiguousarray(np.asarray(wo, np.float32).T).astype(bf),
        "bq_col": np.ascontiguousarray(
            np.asarray(bq, np.float32).reshape(DT, P).T),
        "bk_col": np.ascontiguousarray(
            np.asarray(bk, np.float32).reshape(DT, P).T),
        "bv_bc": np.ascontiguousarray(
            np.broadcast_to(np.asarray(bv, np.float32), (P, D))),
        "bo_row": np.asarray(bo, np.float32).reshape(1, D).copy(),
        "ra0_bc": np.ascontiguousarray(
            np.broadcast_to(np.asarray(ra0, np.float32), (P, D))),
        "rb0_bc": np.ascontiguousarray(
            np.broadcast_to(np.asarray(rb0, np.float32), (P, D))),
        "ra1_bc": np.ascontiguousarray(
            np.broadcast_to(np.asarray(ra1, np.float32), (P, D))),
        "rb1_bc": np.ascontiguousarray(
            np.broadcast_to(np.asarray(rb1, np.float32), (P, D))),
        "a0_bc": np.ascontiguousarray(
            np.broadcast_to(np.asarray(a0, np.float32), (P, D))),
        "b0_bc": np.ascontiguousarray(
            np.broadcast_to(np.asarray(b0, np.float32), (P, D))),
        "ones_in": np.ones((1, P), np.float32),
        "ident_in": np.eye(P, dtype=np.float32).astype(bf),
    }
    x = np.asarray(x, np.float32)
    in_maps = []
    for c in range(8):
        b, q0 = c // GRP, (c % GRP) * OWN
        m = dict(base)
        # rotate tokens so this core's output shard sits at rows 0..OWN
        m["xs"] = np.ascontiguousarray(
            np.concatenate([x[b, q0:], x[b, :q0]], axis=0))
        in_maps.append(m)
    return in_maps


def kernel(**inputs):
    nc = _get_nc()
    in_maps = _prep_inputs(**inputs)
    res = run_bass_kernel_spmd(nc, in_maps, list(range(8)))
    B = inputs["x"].shape[0]
    out = np.empty((B, S, D), np.float32)
    for c in range(8):
        b, q0 = c // GRP, (c % GRP) * OWN
        out[b, q0:q0 + OWN, :] = res.results[c]["out"]
    return out


if __name__ == "__main__":
    rng = np.random.default_rng(0)
    ins = {
        "x": rng.standard_normal((2, S, D)).astype(np.float32),
        "a0": np.ones(D, np.float32), "b0": np.zeros(D, np.float32),
        "ra0": np.ones(D, np.float32), "rb0": np.zeros(D, np.float32),
        "ra1": np.ones(D, np.float32), "rb1": np.zeros(D, np.float32),
        "wq": (rng.standard_normal((D, D)) * 0.02).astype(np.float32),
        "bq": np.zeros(D, np.float32),
        "wk": (rng.standard_normal((D, D)) * 0.02).astype(np.float32),
        "bk": np.zeros(D, np.float32),
        "wv": (rng.standard_normal((D, D)) * 0.02).astype(np.float32),
        "bv": np.zeros(D, np.float32),
        "wo": (rng.standard_normal((D, D)) * 0.02).astype(np.float32),
        "bo": np.zeros(D, np.float32),
    }
    out = kernel(**ins)
    print("kernel ran, out shape", out.shape, out.dtype)


# revision 18
# speedup vs baseline: 2.1313x; 1.0713x over previous
"""Trainium2 Bass kernel for the nn_EncoderBlock problem.

Full inputs in, full output out. 8-way SPMD: cores 0-3 handle batch 0,
cores 4-7 batch 1. Within each 4-core batch group, BOTH blocks are
query-sharded 4 ways (1024 owned rows per core). Between the blocks, the
post-LayerNorm activations zb2 = LN(LN(x2)) (bf16, the exact values the
matmuls would consume anyway) are exchanged with a DRAM AllGather over
replica groups [[0..3],[4..7]] so every core can build block-2 K/V for
the full sequence. Block-2 keys are consumed in global row order (the
gather's order); attention is permutation-invariant over keys, so this
coexists with each core's rotated local query order.

All 8 cores run the SAME program: the host rotates each core's token
order by its query offset -- "queries 0..1023" on the device are exactly
the core's own output shard, while the key set stays complete.

Per block: LN(LN(x)) -> QKV projections -> per-head attention with
scores kept transposed [keys, queries] so softmax's exp doubles as the
PSUM->SBUF evacuation on the scalar engine (no max-subtraction needed:
|scores| < 2), P*V via a ones-augmented V (M=65) so the softmax
denominator falls out of the same matmul, normalization via a K=1
outer-product broadcast matmul, output projection with bias folded in
as a K=1 matmul, residual add. bf16 matmul operands, f32 accumulation,
f32 residual stream. Score matmuls are row-packed two heads at a time
(K=64 pairs on array rows 0-63/64-127).
"""

import sys

sys.path.insert(0, "/opt/trn_rl_repo")

import numpy as np
import ml_dtypes

import bass_rust
import concourse.bass as bass
import concourse.tile as tile
from concourse import mybir
from concourse.bass_utils import run_bass_kernel_spmd

F32 = mybir.dt.float32
F32R = mybir.dt.float32r
BF16 = mybir.dt.bfloat16
FP8 = mybir.dt.float8e4
AF = mybir.ActivationFunctionType
ALU = mybir.AluOpType
DR = mybir.MatmulPerfMode.DoubleRow

P = 128
D = 384
H = 6
DK = 64
DT = D // P          # 3 D-chunks of 128
S = 4096             # full sequence per batch
NTS = S // P         # 32 token tiles of 128
NKC = S // P         # 32 key chunks of 128
OWN = 1024           # query tokens owned per core (both blocks)
NOT = OWN // P       # 8 owned token tiles
GRP = 4              # cores per batch group
EPS = 1e-6
QT = 512             # query tile (free dim of score matmuls)
NQT = OWN // QT      # 2 query tiles per block
KCG = 3              # key chunks per exp group (3*512 = 1536 psum cols)
VROW = H * (DK + 1)  # 390: per-kc row of V_aug (64 data cols + ones col/head)
VROW2 = 400          # padded to a 16-byte-multiple stride for DoubleRow
CC_GROUPS = [[0, 1, 2, 3], [4, 5, 6, 7]]

# ---------------------------------------------------------------------------
# walrus in this container caps sync-waits per instruction (1 for most,
# 0 for DMA-transpose). Hoist excess waits onto same-engine NoOps.
_WAIT_LIMIT_BY_TYPE = {"InstDmaTransposeAnt": 0}
_wfix_ctr = [0]


def _fix_sync_waits(nc):
    for f in nc.m.functions:
        for bb in f.blocks:
            out = []
            changed = False
            for ins in bb.instructions:
                si = ins.sync_info
                waits = list(si.on_wait) if si is not None else []
                limit = _WAIT_LIMIT_BY_TYPE.get(type(ins).__name__, 1)
                if len(waits) > limit:
                    keep, hoist = waits[:limit], waits[limit:]
                    for w in hoist:
                        _wfix_ctr[0] += 1
                        nop = mybir.InstNoOp(
                            name=f"WFIX-{_wfix_ctr[0]}", engine=ins.engine
                        )
                        nop.sync_info = bass_rust.SyncInfo(on_wait=[w], on_update=[])
                        out.append(nop)
                    ins.sync_info = bass_rust.SyncInfo(
                        on_wait=keep, on_update=list(si.on_update)
                    )
                    changed = True
                out.append(ins)
            if changed:
                bb.instructions = out


def _rows(dram_ap, row0, nrows):
    """[nrows, D] rows of a [*, D] DRAM tensor as a DMA AP."""
    return bass.AP(tensor=dram_ap.tensor,
                   offset=dram_ap.offset + row0 * D,
                   ap=[[D, nrows], [1, D]])


# ---------------------------------------------------------------------------
def _emit_ln_tile(nc, pools, C, x_src_d, n, blk):
    """LN(LN(x)) for one 128-token tile -> zb (bf16). Pure DVE + tiny ACT,
    zero PSUM usage, so these interleave into attention without stalling
    the in-order PE stream.

    When every LN affine is identity (the staged problem: a=1, b=0),
    LN(LN(x)) folds exactly to (x - m) / (s*(1+eps) + eps^2): the inner
    LN's output has mean 0 and std s/(s+eps), so the outer divide just
    rescales the same centered row. One bn_stats + one tensor_scalar
    per tile instead of two full passes."""
    work = pools["work"]

    if C["identity_ln"]:
        xt = work.tile([P, D], F32, tag="x_ln", name=f"xln{blk}_{n}")
        nc.sync.dma_start(out=xt[:, :], in_=_rows(x_src_d, n * P, P))
        mv = work.tile([P, 6 + 2], F32, tag="ln_mv", name=f"mvf_{blk}_{n}")
        nc.vector.bn_stats(out=mv[:, 0:6], in_=xt[:, :])
        nc.vector.bn_aggr(out=mv[:, 6:8], in_=mv[:, 0:6])
        r = work.tile([P, 1], F32, tag="ln_r", name=f"rf_{blk}_{n}")
        nc.scalar.activation(out=r[:, :], in_=mv[:, 7:8], func=AF.Ln,
                             scale=float(D) / float(D - 1))
        nc.scalar.activation(out=r[:, :], in_=r[:, :], func=AF.Exp,
                             scale=0.5)
        nc.vector.tensor_scalar(out=r[:, :], in0=r[:, :],
                                scalar1=1.0 + EPS, scalar2=EPS * EPS,
                                op0=ALU.mult, op1=ALU.add)
        nc.vector.reciprocal(out=r[:, :], in_=r[:, :])
        zb = work.tile([P, D], BF16, tag=f"zb{blk}",
                       bufs=(NOT if blk == 1 else 4),
                       name=f"zb_{blk}_{n}")
        nc.vector.tensor_scalar(
            out=zb[:, :], in0=xt[:, :],
            scalar1=mv[:, 6:7], scalar2=r[:, 0:1],
            op0=ALU.subtract, op1=ALU.mult)
        return zb

    def _ln_pass(src_ap, m_ra, m_rb, dst_ap, uid):
        mv = work.tile([P, 6 + 2], F32, tag="ln_mv", name=f"mv_{uid}")
        nc.vector.bn_stats(out=mv[:, 0:6], in_=src_ap)
        nc.vector.bn_aggr(out=mv[:, 6:8], in_=mv[:, 0:6])
        r = work.tile([P, 1], F32, tag="ln_r", name=f"r_{uid}")
        nc.scalar.activation(out=r[:, :], in_=mv[:, 7:8], func=AF.Ln,
                             scale=float(D) / float(D - 1))
        nc.scalar.activation(out=r[:, :], in_=r[:, :], func=AF.Exp,
                             scale=0.5)
        nc.vector.tensor_scalar_add(out=r[:, :], in0=r[:, :], scalar1=EPS)
        nc.vector.reciprocal(out=r[:, :], in_=r[:, :])
        t = work.tile([P, D], F32, tag="ln_t", name=f"t_{uid}")
        nc.vector.tensor_scalar(
            out=t[:, :], in0=src_ap,
            scalar1=mv[:, 6:7], scalar2=r[:, 0:1],
            op0=ALU.subtract, op1=ALU.mult)
        nc.vector.tensor_mul(out=t[:, :], in0=t[:, :], in1=m_ra[:, :])
        nc.vector.tensor_add(out=dst_ap, in0=t[:, :], in1=m_rb[:, :])

    xt = work.tile([P, D], F32, tag="x_ln", name=f"xln{blk}_{n}")
    nc.sync.dma_start(out=xt[:, :], in_=_rows(x_src_d, n * P, P))
    yt = work.tile([P, D], F32, tag="y1", name=f"y1_{blk}_{n}")
    _ln_pass(xt[:, :], C["ra_bc"], C["rb_bc"], yt[:, :], f"{blk}_{n}a")
    zb = work.tile([P, D], BF16, tag=f"zb{blk}",
                   bufs=(NOT if blk == 1 else 4),
                   name=f"zb_{blk}_{n}")
    _ln_pass(yt[:, :], C["a0_bc"], C["b0_bc"], zb[:, :], f"{blk}_{n}b")
    return zb


def _transpose_tile(nc, pools, C, zb_ap, dst, n, uid):
    """3 PE transposes of a [128, D] bf16 tile into dst[:, dt, n*P:...]."""
    psB = pools["psB"]
    for dt_ in range(DT):
        tp = psB.tile([P, P], BF16, tag="acc", name=f"tp{uid}_{n}_{dt_}")
        nc.tensor.transpose(out=tp[:, 0:P],
                            in_=zb_ap[:, dt_ * P:(dt_ + 1) * P],
                            identity=C["ident"][:, :])
        # evacuate on ACT: it is idle in the LN valley / boundary,
        # while DVE is the critical engine there
        nc.scalar.copy(out=dst[:, dt_, n * P:(n + 1) * P],
                       in_=tp[:, 0:P])


# ---------------------------------------------------------------------------
def _build_block(nc, pools, C, x_src_d, out_d, blk,
                 zb_dram=None, premade_zb=None, after_qt=None):
    """One residual MSA block, query-sharded to OWN rows.

    x_src_d: DRAM AP [*, D] f32 -- local rows (rotated order); rows
             0..OWN are this core's queries and residual base.
    out_d:   DRAM AP [OWN, D] f32 -- gets x_src[0:OWN] + MSA(...)[0:OWN]
    zb_dram: if set (block 2), [S, D] bf16 DRAM with LN(LN(x2)) for the
             WHOLE sequence in global order -- K/V source. If None
             (block 1), LN chains run locally on x_src_d for all S rows.
    premade_zb: block 2 only -- the 8 locally-computed own zb tiles
             (rotated order), transposed into zTq for the Q projection.
    """
    work, psA, psB, ste_pool, otp = (pools[k] for k in
                                     ("work", "psA", "psB", "ste", "ot"))

    qt_sb, kt_sb, v_aug = C["qt"], C["kt"], C["v_aug"]
    zT = C["zT"]

    def _q_proj(src):
        for dt_ in range(DT):
            for ntk in range(OWN // QT):
                ps = psB.tile([P, QT], F32, tag="acc",
                              name=f"pq{blk}_{dt_}_{ntk}")
                for ki in range(DT):
                    nc.tensor.matmul(
                        ps[:, :],
                        lhsT=C["wqT"][:, ki, dt_ * P:(dt_ + 1) * P],
                        rhs=src[:, ki, ntk * QT:(ntk + 1) * QT],
                        start=(ki == 0), stop=(ki == DT - 1))
                nc.vector.tensor_scalar(
                    out=qt_sb[:, dt_, ntk * QT:(ntk + 1) * QT], in0=ps[:, :],
                    scalar1=C["bq_col"][:, dt_:dt_ + 1], scalar2=None,
                    op0=ALU.add)

    def _kv_tiles(n0, n1):
        """zT (from LN or gather) + K-proj + V_aug for token tiles n0..n1."""
        for n in range(n0, n1):
            if zb_dram is not None:
                half = zb_dram[n // (NTS // 2)]
                zb = work.tile([P, D], BF16, tag="zb_g", name=f"zbg_{n}")
                nc.sync.dma_start(
                    out=zb[:, :],
                    in_=_rows(half, (n % (NTS // 2)) * P, P))
            else:
                zb = _emit_ln_tile(nc, pools, C, x_src_d, n, blk)
            _transpose_tile(nc, pools, C, zb[:, :], zT, n, f"z{blk}")
        for dt_ in range(DT):
            for ntk in range(n0 * P // QT, n1 * P // QT):
                ps = psB.tile([P, QT], F32, tag="acc",
                              name=f"pk{blk}_{dt_}_{ntk}")
                for ki in range(DT):
                    nc.tensor.matmul(
                        ps[:, :],
                        lhsT=C["wkT"][:, ki, dt_ * P:(dt_ + 1) * P],
                        rhs=zT[:, ki, ntk * QT:(ntk + 1) * QT],
                        start=(ki == 0), stop=(ki == DT - 1))
                nc.vector.tensor_scalar(
                    out=kt_sb[:, dt_, ntk * QT:(ntk + 1) * QT], in0=ps[:, :],
                    scalar1=C["bk_col"][:, dt_:dt_ + 1], scalar2=None,
                    op0=ALU.add)
        for n in range(n0, n1):
            ps = psB.tile([P, QT], F32, tag="acc", name=f"v{blk}_{n}")
            for ki in range(DT):
                nc.tensor.matmul(
                    ps[:, :D],
                    lhsT=zT[:, ki, n * P:(n + 1) * P],
                    rhs=C["wvT"][:, ki, :],
                    start=(ki == 0), stop=(ki == DT - 1))
            # ones-augmented layout; ones at j=DK persist from memset
            nc.vector.tensor_tensor(
                out=v_aug[:, n, 0:VROW].rearrange(
                    "p (h j) -> p h j", h=H, j=DK + 1)[:, :, 0:DK],
                in0=ps[:, :D].rearrange("p (h j) -> p h j", h=H, j=DK),
                in1=C["bv_bc"][:, :].rearrange("p (h j) -> p h j", h=H, j=DK),
                op=ALU.add)

    if premade_zb is None:
        # block 1: all LN local; zT rows 0..OWN are exactly the own
        # queries. 8-tile chunks let K/V projections interleave with the
        # DVE-bound LN stream instead of waiting for all 32 tiles.
        for c in range(0, NTS, 8):
            _kv_tiles(c, c + 8)
        _q_proj(zT)
    else:
        # block 2: emit gather0-dependent K/V first (ready the moment
        # block 1 ends), then the own-LN-dependent Q path, then the
        # gather1-dependent second half.
        _kv_tiles(0, NTS // 2)
        zTq = C["zTq"]
        for n in range(NOT):
            _transpose_tile(nc, pools, C, premade_zb[n][:, :], zTq, n,
                            f"q{blk}")
        _q_proj(zTq)
        _kv_tiles(NTS // 2, NTS)

    # ---- attention (own queries) + per-qt output projection ----
    n_groups = (NKC + KCG - 1) // KCG
    for ntk in range(NQT):
        ot = otp.tile([P, DT, QT], BF16, tag="ot", name=f"ot{blk}_{ntk}")
        for hp in range(DT):
            pv = [psB.tile([P, QT], F32, tag="acc",
                           name=f"pv{blk}_{ntk}_{hp}_{i}") for i in range(2)]
            for g in range(n_groups):
                kcs = list(range(g * KCG, min(NKC, (g + 1) * KCG)))
                w = len(kcs) * QT
                for half in range(2):   # head pair on partitions 0-63/64-127
                    lo = half * DK
                    st = psA.tile([P, KCG * QT], F32, tag="st",
                                  name=f"st{blk}_{ntk}_{hp}_{g}_{half}")
                    for j, kc in enumerate(kcs):
                        nc.tensor.matmul(
                            st[:, j * QT:(j + 1) * QT],
                            lhsT=kt_sb[lo:lo + DK, hp, kc * P:(kc + 1) * P],
                            rhs=qt_sb[lo:lo + DK, hp,
                                      ntk * QT:(ntk + 1) * QT],
                            start=True, stop=True)
                    ste = ste_pool.tile([P, KCG * QT], FP8, tag="ste",
                                        name=f"se{blk}_{ntk}_{hp}_{g}_{half}")
                    nc.scalar.activation(out=ste[:, :w], in_=st[:, :w],
                                         func=AF.Exp, scale=1.0 / 8.0)
                    h = 2 * hp + half
                    # P*V: fp8 DoubleRow over kc pairs (2 key chunks per
                    # matmul at 0.5 cyc/row), odd remainder plain fp8.
                    j = 0
                    while j < len(kcs):
                        kc = kcs[j]
                        if j + 1 < len(kcs):
                            nc.tensor.matmul(
                                pv[half][0:DK + 1, :],
                                lhsT=v_aug[:, kc:kc + 2,
                                           h * (DK + 1):(h + 1) * (DK + 1)],
                                rhs=ste[:, j * QT:(j + 2) * QT].rearrange(
                                    "p (k q) -> p k q", k=2, q=QT),
                                start=(kc == 0), stop=(kc + 1 == NKC - 1),
                                perf_mode=DR, skip_group_check=True)
                            j += 2
                        else:
                            nc.tensor.matmul(
                                pv[half][0:DK + 1, :],
                                lhsT=v_aug[:, kc,
                                           h * (DK + 1):(h + 1) * (DK + 1)],
                                rhs=ste[:, j * QT:(j + 1) * QT],
                                start=(kc == 0), stop=(kc == NKC - 1),
                                skip_group_check=True)
                            j += 1
            for half in range(2):
                lo = half * DK
                r_row = work.tile([1, QT], F32R, tag="r_row",
                                  name=f"rr{blk}_{ntk}_{hp}_{half}")
                with nc.allow_low_precision(
                        reason="f32r broadcast of softmax denom"):
                    nc.vector.reciprocal(
                        out=r_row[:, :], in_=pv[half][DK:DK + 1, :])
                r_bc = psA.tile([P, KCG * QT], F32, tag="st",
                                name=f"rb{blk}_{ntk}_{hp}_{half}")
                nc.tensor.matmul(
                    r_bc[0:DK, 0:QT],
                    lhsT=C["ones"][0:1, 0:DK],
                    rhs=r_row[0:1, :],
                    start=True, stop=True)
                r_sb = work.tile([DK, QT], F32, tag="r_sb",
                                 name=f"rs{blk}_{ntk}_{hp}_{half}")
                nc.vector.tensor_copy(out=r_sb[:, :], in_=r_bc[0:DK, 0:QT])
                nc.vector.tensor_tensor(
                    out=ot[lo:lo + DK, hp, :],
                    in0=pv[half][0:DK, :], in1=r_sb[:, :], op=ALU.mult)
        # output projection + bias + residual for this query tile
        for c4 in range(QT // P):
            tok = ntk * QT + c4 * P
            ps = psB.tile([P, QT], F32, tag="acc",
                          name=f"o{blk}_{ntk}_{c4}")
            for ki in range(DT):
                nc.tensor.matmul(
                    ps[:, :D],
                    lhsT=ot[:, ki, c4 * P:(c4 + 1) * P],
                    rhs=C["woT"][:, ki, :],
                    start=(ki == 0), stop=False)
            nc.tensor.matmul(
                ps[:, :D],
                lhsT=C["ones"][0:1, 0:P],
                rhs=C["bo_row"][0:1, :],
                start=False, stop=True, skip_group_check=True)
            xr = work.tile([P, D], F32, tag="x_res",
                           name=f"xr{blk}_{ntk}_{c4}")
            nc.sync.dma_start(out=xr[:, :], in_=_rows(x_src_d, tok, P))
            xo = work.tile([P, D], F32, tag="x_out",
                           name=f"xo{blk}_{ntk}_{c4}")
            nc.vector.tensor_tensor(
                out=xo[:, :], in0=ps[:, :D], in1=xr[:, :], op=ALU.add)
            nc.sync.dma_start(out=_rows(out_d, tok, P), in_=xo[:, :])
        if after_qt is not None:
            after_qt(ntk)


def _build_program(identity_ln):
    nc = bass.Bass("TRN2", target_bir_lowering=False, debug=False,
                   num_devices=8)

    di = {}
    di["xs"] = nc.dram_tensor("xs", [S, D], F32, kind="ExternalInput")
    for w in ("wqT", "wkT", "wvT", "woT"):
        di[w] = nc.dram_tensor(w, [D, D], BF16, kind="ExternalInput")
    di["bq_col"] = nc.dram_tensor("bq_col", [P, DT], F32, kind="ExternalInput")
    di["bk_col"] = nc.dram_tensor("bk_col", [P, DT], F32, kind="ExternalInput")
    di["bv_bc"] = nc.dram_tensor("bv_bc", [P, D], F32, kind="ExternalInput")
    di["bo_row"] = nc.dram_tensor("bo_row", [1, D], F32R, kind="ExternalInput")
    for w in ("ra0_bc", "rb0_bc", "ra1_bc", "rb1_bc", "a0_bc", "b0_bc"):
        di[w] = nc.dram_tensor(w, [P, D], F32, kind="ExternalInput")
    di["ones_in"] = nc.dram_tensor("ones_in", [1, P], F32R,
                                   kind="ExternalInput")
    di["ident_in"] = nc.dram_tensor("ident_in", [P, P], BF16,
                                    kind="ExternalInput")
    out_d = nc.dram_tensor("out", [OWN, D], F32, kind="ExternalOutput")
    x2_d = nc.dram_tensor("x2buf", [OWN, D], F32)        # internal
    # split gather: half h carries each member's own rows [h*512:(h+1)*512];
    # separate tensors so first-half consumers never falsely depend on the
    # second collective.
    gin_d = [nc.dram_tensor(f"gin{h}", [OWN // 2, D], BF16) for h in range(2)]
    gout_d = [nc.dram_tensor(f"gout{h}", [S // 2, D], BF16) for h in range(2)]

    with tile.TileContext(nc) as tc:
        with tc.tile_pool(name="const", bufs=1) as const, \
             tc.tile_pool(name="work", bufs=3) as work, \
             tc.tile_pool(name="ot", bufs=2) as otp, \
             tc.tile_pool(name="ste", bufs=6) as ste_pool, \
             tc.tile_pool(name="psA", bufs=2, space="PSUM") as psA, \
             tc.tile_pool(name="psB", bufs=2, space="PSUM") as psB:

            pools = {"work": work, "psA": psA, "psB": psB,
                     "ste": ste_pool, "ot": otp}

            C = {}
            for wname in ("wqT", "wkT", "wvT", "woT"):
                C[wname] = const.tile([P, DT, D], BF16, name=wname)
                nc.sync.dma_start(
                    out=C[wname][:, :, :],
                    in_=di[wname][:].rearrange("(d p) e -> p d e", p=P))
            for wname in ("bq_col", "bk_col", "bv_bc"):
                C[wname] = const.tile(list(di[wname].shape), F32, name=wname)
                nc.sync.dma_start(out=C[wname][:], in_=di[wname][:])
            C["bo_row"] = const.tile([1, D], F32R, name="bo_row")
            nc.sync.dma_start(out=C["bo_row"][:], in_=di["bo_row"][:])
            for wname in ("ra0_bc", "rb0_bc", "ra1_bc", "rb1_bc",
                          "a0_bc", "b0_bc"):
                C[wname] = const.tile([P, D], F32, name=wname)
                nc.sync.dma_start(out=C[wname][:, :], in_=di[wname][:])
            C["ones"] = const.tile([1, P], F32R, name="ones")
            nc.sync.dma_start(out=C["ones"][:, :], in_=di["ones_in"][:])
            C["ident"] = const.tile([P, P], BF16, name="ident")
            nc.sync.dma_start(out=C["ident"][:, :], in_=di["ident_in"][:])

            C["zT"] = const.tile([P, DT, S], BF16, name="zT")
            C["zTq"] = const.tile([P, DT, OWN], BF16, name="zTq")
            C["qt"] = const.tile([P, DT, OWN], BF16, name="qt")
            C["kt"] = const.tile([P, DT, S], BF16, name="kt")
            C["v_aug"] = const.tile([P, NTS, VROW2], FP8, name="v_aug")
            nc.vector.memset(C["v_aug"][:, :, :], 1.0)

            C["identity_ln"] = identity_ln
            C1 = dict(C)
            C1["ra_bc"], C1["rb_bc"] = C["ra0_bc"], C["rb0_bc"]
            C2 = dict(C)
            C2["ra_bc"], C2["rb_bc"] = C["ra1_bc"], C["rb1_bc"]

            # block-2 LN of OWN rows interleaves into block-1 attention:
            # as each query tile's x2 rows land, run the pure-DVE LN chain,
            # stage the bf16 result into that half's gather input, and kick
            # the half-AllGather that publishes it group-wide. The first
            # gather runs concurrently with block-1's second query tile.
            zb2 = [None] * NOT

            def _b2_ln(ntk):
                for n in range(ntk * 4, ntk * 4 + 4):
                    zb2[n] = _emit_ln_tile(nc, pools, C2, x2_d[:], n, 1)
                    nc.sync.dma_start(
                        out=_rows(gin_d[ntk][:], (n % 4) * P, P),
                        in_=zb2[n][:, :])
                nc.gpsimd.collective_compute(
                    "AllGather", mybir.AluOpType.bypass,
                    replica_groups=CC_GROUPS,
                    ins=[gin_d[ntk][:].opt()], outs=[gout_d[ntk][:].opt()])

            _build_block(nc, pools, C1, di["xs"][:], x2_d[:], 0,
                         after_qt=_b2_ln)
            _build_block(nc, pools, C2, x2_d[:], out_d[:], 1,
                         zb_dram=[g[:] for g in gout_d], premade_zb=zb2)

    _fix_sync_waits(nc)
    return nc


_NC_CACHE = {}


def _get_nc(identity_ln=True):
    if identity_ln not in _NC_CACHE:
        _NC_CACHE[identity_ln] = _build_program(identity_ln)
    return _NC_CACHE[identity_ln]


def _prep_inputs(x, a0, b0, ra0, rb0, ra1, rb1,
                 wq, bq, wk, bk, wv, bv, wo, bo):
    bf = ml_dtypes.bfloat16
    base = {
        "wqT": np.ascontiguousarray(np.asarray(wq, np.float32).T).astype(bf),
        "wkT": np.ascontiguousarray(np.asarray(wk, np.float32).T).astype(bf),
        "wvT": np.ascontiguousarray(np.asarray(wv, np.float32).T).astype(bf),
        "woT": np.ascontiguousarray(np.asarray(wo, np.float32).T).astype(bf),
        "bq_col": np.ascontiguousarray(
            np.asarray(bq, np.float32).reshape(DT, P).T),
        "bk_col": np.ascontiguousarray(
            np.asarray(bk, np.float32).reshape(DT, P).T),
        "bv_bc": np.ascontiguousarray(
            np.broadcast_to(np.asarray(bv, np.float32), (P, D))),
        "bo_row": np.asarray(bo, np.float32).reshape(1, D).copy(),
        "ra0_bc": np.ascontiguousarray(
            np.broadcast_to(np.asarray(ra0, np.float32), (P, D))),
        "rb0_bc": np.ascontiguousarray(
            np.broadcast_to(np.asarray(rb0, np.float32), (P, D))),
        "ra1_bc": np.ascontiguousarray(
            np.broadcast_to(np.asarray(ra1, np.float32), (P, D))),
        "rb1_bc": np.ascontiguousarray(
            np.broadcast_to(np.asarray(rb1, np.float32), (P, D))),
        "a0_bc": np.ascontiguousarray(
            np.broadcast_to(np.asarray(a0, np.float32), (P, D))),
        "b0_bc": np.ascontiguousarray(
            np.broadcast_to(np.asarray(b0, np.float32), (P, D))),
        "ones_in": np.ones((1, P), np.float32),
        "ident_in": np.eye(P, dtype=np.float32).astype(bf),
    }
    x = np.asarray(x, np.float32)
    in_maps = []
    for c in range(8):
        b, q0 = c // GRP, (c % GRP) * OWN
        m = dict(base)
        # rotate tokens so this core's output shard sits at rows 0..OWN
        m["xs"] = np.ascontiguousarray(
            np.concatenate([x[b, q0:], x[b, :q0]], axis=0))
        in_maps.append(m)
    return in_maps


def kernel(**inputs):
    identity_ln = all(
        bool(np.all(np.asarray(inputs[k], np.float32) == v))
        for k, v in (("a0", 1.0), ("b0", 0.0), ("ra0", 1.0), ("rb0", 0.0),
                     ("ra1", 1.0), ("rb1", 0.0)))
    nc = _get_nc(identity_ln)
    in_maps = _prep_inputs(**inputs)
    res = run_bass_kernel_spmd(nc, in_maps, list(range(8)))
    B = inputs["x"].shape[0]
    out = np.empty((B, S, D), np.float32)
    for c in range(8):
        b, q0 = c // GRP, (c % GRP) * OWN
        out[b, q0:q0 + OWN, :] = res.results[c]["out"]
    return out


if __name__ == "__main__":
    rng = np.random.default_rng(0)
    ins = {
        "x": rng.standard_normal((2, S, D)).astype(np.float32),
        "a0": np.ones(D, np.float32), "b0": np.zeros(D, np.float32),
        "ra0": np.ones(D, np.float32), "rb0": np.zeros(D, np.float32),
        "ra1": np.ones(D, np.float32), "rb1": np.zeros(D, np.float32),
        "wq": (rng.standard_normal((D, D)) * 0.02).astype(np.float32),
        "bq": np.zeros(D, np.float32),
        "wk": (rng.standard_normal((D, D)) * 0.02).astype(np.float32),
        "bk": np.zeros(D, np.float32),
        "wv": (rng.standard_normal((D, D)) * 0.02).astype(np.float32),
        "bv": np.zeros(D, np.float32),
        "wo": (rng.standard_normal((D, D)) * 0.02).astype(np.float32),
        "bo": np.zeros(D, np.float32),
    }
    out = kernel(**ins)
    print("kernel ran, out shape", out.shape, out.dtype)


# revision 28
# speedup vs baseline: 2.1582x; 1.0126x over previous
"""Trainium2 Bass kernel for the nn_EncoderBlock problem.

Full inputs in, full output out. 8-way SPMD: cores 0-3 handle batch 0,
cores 4-7 batch 1. Within each 4-core batch group, BOTH blocks are
query-sharded 4 ways (1024 owned rows per core). Between the blocks, the
post-LayerNorm activations zb2 = LN(LN(x2)) (bf16, the exact values the
matmuls would consume anyway) are exchanged with a DRAM AllGather over
replica groups [[0..3],[4..7]] so every core can build block-2 K/V for
the full sequence. Block-2 keys are consumed in global row order (the
gather's order); attention is permutation-invariant over keys, so this
coexists with each core's rotated local query order.

All 8 cores run the SAME program: the host rotates each core's token
order by its query offset -- "queries 0..1023" on the device are exactly
the core's own output shard, while the key set stays complete.

Per block: LN(LN(x)) -> QKV projections -> per-head attention with
scores kept transposed [keys, queries] so softmax's exp doubles as the
PSUM->SBUF evacuation on the scalar engine (no max-subtraction needed:
|scores| < 2), P*V via a ones-augmented V (M=65) so the softmax
denominator falls out of the same matmul, normalization via a K=1
outer-product broadcast matmul, output projection with bias folded in
as a K=1 matmul, residual add. bf16 matmul operands, f32 accumulation,
f32 residual stream. Score matmuls are row-packed two heads at a time
(K=64 pairs on array rows 0-63/64-127).
"""

import sys

sys.path.insert(0, "/opt/trn_rl_repo")

import numpy as np
import ml_dtypes

import bass_rust
import concourse.bass as bass
import concourse.tile as tile
from concourse import mybir
from concourse.bass_utils import run_bass_kernel_spmd

F32 = mybir.dt.float32
F32R = mybir.dt.float32r
BF16 = mybir.dt.bfloat16
FP8 = mybir.dt.float8e4
AF = mybir.ActivationFunctionType
ALU = mybir.AluOpType
DR = mybir.MatmulPerfMode.DoubleRow

P = 128
D = 384
H = 6
DK = 64
DT = D // P          # 3 D-chunks of 128
S = 4096             # full sequence per batch
NTS = S // P         # 32 token tiles of 128
NKC = S // P         # 32 key chunks of 128
OWN = 1024           # query tokens owned per core (both blocks)
NOT = OWN // P       # 8 owned token tiles
GRP = 4              # cores per batch group
EPS = 1e-6
QT = 512             # query tile (free dim of score matmuls)
NQT = OWN // QT      # 2 query tiles per block
KCG = 3              # key chunks per exp group (3*512 = 1536 psum cols)
VROW = H * (DK + 1)  # 390: per-kc row of V_aug (64 data cols + ones col/head)
VROW2 = 400          # padded to a 16-byte-multiple stride for DoubleRow
CC_GROUPS = [[0, 1, 2, 3], [4, 5, 6, 7]]

# ---------------------------------------------------------------------------
# walrus in this container caps sync-waits per instruction (1 for most,
# 0 for DMA-transpose). Hoist excess waits onto same-engine NoOps.
_WAIT_LIMIT_BY_TYPE = {"InstDmaTransposeAnt": 0}
_wfix_ctr = [0]


def _fix_sync_waits(nc):
    for f in nc.m.functions:
        for bb in f.blocks:
            out = []
            changed = False
            for ins in bb.instructions:
                si = ins.sync_info
                waits = list(si.on_wait) if si is not None else []
                limit = _WAIT_LIMIT_BY_TYPE.get(type(ins).__name__, 1)
                if len(waits) > limit:
                    keep, hoist = waits[:limit], waits[limit:]
                    for w in hoist:
                        _wfix_ctr[0] += 1
                        nop = mybir.InstNoOp(
                            name=f"WFIX-{_wfix_ctr[0]}", engine=ins.engine
                        )
                        nop.sync_info = bass_rust.SyncInfo(on_wait=[w], on_update=[])
                        out.append(nop)
                    ins.sync_info = bass_rust.SyncInfo(
                        on_wait=keep, on_update=list(si.on_update)
                    )
                    changed = True
                out.append(ins)
            if changed:
                bb.instructions = out


def _rows(dram_ap, row0, nrows):
    """[nrows, D] rows of a [*, D] DRAM tensor as a DMA AP."""
    return bass.AP(tensor=dram_ap.tensor,
                   offset=dram_ap.offset + row0 * D,
                   ap=[[D, nrows], [1, D]])


# ---------------------------------------------------------------------------
def _emit_ln_tile(nc, pools, C, x_src_d, n, blk):
    """LN(LN(x)) for one 128-token tile -> zb (bf16). Pure DVE + tiny ACT,
    zero PSUM usage, so these interleave into attention without stalling
    the in-order PE stream.

    When every LN affine is identity (the staged problem: a=1, b=0),
    LN(LN(x)) folds exactly to (x - m) / (s*(1+eps) + eps^2): the inner
    LN's output has mean 0 and std s/(s+eps), so the outer divide just
    rescales the same centered row. One bn_stats + one tensor_scalar
    per tile instead of two full passes."""
    work = pools["work"]

    if C["identity_ln"]:
        xt = work.tile([P, D], F32, tag="x_ln", name=f"xln{blk}_{n}")
        nc.sync.dma_start(out=xt[:, :], in_=_rows(x_src_d, n * P, P))
        mv = work.tile([P, 6 + 2], F32, tag="ln_mv", name=f"mvf_{blk}_{n}")
        nc.vector.bn_stats(out=mv[:, 0:6], in_=xt[:, :])
        nc.vector.bn_aggr(out=mv[:, 6:8], in_=mv[:, 0:6])
        r = work.tile([P, 1], F32, tag="ln_r", name=f"rf_{blk}_{n}")
        nc.scalar.activation(out=r[:, :], in_=mv[:, 7:8], func=AF.Sqrt,
                             scale=float(D) / float(D - 1))
        nc.vector.tensor_scalar(out=r[:, :], in0=r[:, :],
                                scalar1=1.0 + EPS, scalar2=EPS * EPS,
                                op0=ALU.mult, op1=ALU.add)
        nc.vector.reciprocal(out=r[:, :], in_=r[:, :])
        zb = work.tile([P, D], BF16, tag=f"zb{blk}",
                       bufs=(NOT if blk == 1 else 4),
                       name=f"zb_{blk}_{n}")
        nc.vector.tensor_scalar(
            out=zb[:, :], in0=xt[:, :],
            scalar1=mv[:, 6:7], scalar2=r[:, 0:1],
            op0=ALU.subtract, op1=ALU.mult)
        return zb

    def _ln_pass(src_ap, m_ra, m_rb, dst_ap, uid):
        mv = work.tile([P, 6 + 2], F32, tag="ln_mv", name=f"mv_{uid}")
        nc.vector.bn_stats(out=mv[:, 0:6], in_=src_ap)
        nc.vector.bn_aggr(out=mv[:, 6:8], in_=mv[:, 0:6])
        r = work.tile([P, 1], F32, tag="ln_r", name=f"r_{uid}")
        nc.scalar.activation(out=r[:, :], in_=mv[:, 7:8], func=AF.Ln,
                             scale=float(D) / float(D - 1))
        nc.scalar.activation(out=r[:, :], in_=r[:, :], func=AF.Exp,
                             scale=0.5)
        nc.vector.tensor_scalar_add(out=r[:, :], in0=r[:, :], scalar1=EPS)
        nc.vector.reciprocal(out=r[:, :], in_=r[:, :])
        t = work.tile([P, D], F32, tag="ln_t", name=f"t_{uid}")
        nc.vector.tensor_scalar(
            out=t[:, :], in0=src_ap,
            scalar1=mv[:, 6:7], scalar2=r[:, 0:1],
            op0=ALU.subtract, op1=ALU.mult)
        nc.vector.tensor_mul(out=t[:, :], in0=t[:, :], in1=m_ra[:, :])
        nc.vector.tensor_add(out=dst_ap, in0=t[:, :], in1=m_rb[:, :])

    xt = work.tile([P, D], F32, tag="x_ln", name=f"xln{blk}_{n}")
    nc.sync.dma_start(out=xt[:, :], in_=_rows(x_src_d, n * P, P))
    yt = work.tile([P, D], F32, tag="y1", name=f"y1_{blk}_{n}")
    _ln_pass(xt[:, :], C["ra_bc"], C["rb_bc"], yt[:, :], f"{blk}_{n}a")
    zb = work.tile([P, D], BF16, tag=f"zb{blk}",
                   bufs=(NOT if blk == 1 else 4),
                   name=f"zb_{blk}_{n}")
    _ln_pass(yt[:, :], C["a0_bc"], C["b0_bc"], zb[:, :], f"{blk}_{n}b")
    return zb


def _transpose_tile(nc, pools, C, zb_ap, dst, col0, uid):
    """3 PE transposes of a [128, D] bf16 SBUF tile into dst[:, :, col0:..].
    PSUM->SBUF evacuation on ACT: it is idle in the LN valley / boundary,
    while DVE is the critical engine there. (XBAR DMA transposes are only
    used for the big DRAM gather halves: their 0-wait walrus limit turns
    every dependency into a queue-blocking NoOp, which serializes the LN
    pipeline when used per-tile.)"""
    psB = pools["psB"]
    for dt_ in range(DT):
        tp = psB.tile([P, P], BF16, tag="acc", name=f"tp{uid}_{dt_}")
        nc.tensor.transpose(out=tp[:, 0:P],
                            in_=zb_ap[:, dt_ * P:(dt_ + 1) * P],
                            identity=C["ident"][:, :])
        nc.scalar.copy(out=dst[:, dt_, col0:col0 + P], in_=tp[:, 0:P])


# ---------------------------------------------------------------------------
def _build_block(nc, pools, C, x_src_d, out_d, blk,
                 zb_dram=None, premade_zb=None, after_qt=None):
    """One residual MSA block, query-sharded to OWN rows.

    x_src_d: DRAM AP [*, D] f32 -- local rows (rotated order); rows
             0..OWN are this core's queries and residual base.
    out_d:   DRAM AP [OWN, D] f32 -- gets x_src[0:OWN] + MSA(...)[0:OWN]
    zb_dram: if set (block 2), [S, D] bf16 DRAM with LN(LN(x2)) for the
             WHOLE sequence in global order -- K/V source. If None
             (block 1), LN chains run locally on x_src_d for all S rows.
    premade_zb: block 2 only -- the 8 locally-computed own zb tiles
             (rotated order), transposed into zTq for the Q projection.
    """
    work, psA, psB, ste_pool, otp = (pools[k] for k in
                                     ("work", "psA", "psB", "ste", "ot"))

    qt_sb, kt_sb, v_aug = C["qt"], C["kt"], C["v_aug"]
    zTh = C["zTh"]          # two [P, DT, S//2] halves
    HNT = NTS // 2          # 16 token tiles per half

    def _q_proj(src):
        for dt_ in range(DT):
            for ntk in range(OWN // QT):
                ps = psB.tile([P, QT], F32, tag="acc",
                              name=f"pq{blk}_{dt_}_{ntk}")
                for ki in range(DT):
                    nc.tensor.matmul(
                        ps[:, :],
                        lhsT=C["wqT"][:, ki, dt_ * P:(dt_ + 1) * P],
                        rhs=src[:, ki, ntk * QT:(ntk + 1) * QT],
                        start=(ki == 0), stop=(ki == DT - 1))
                nc.vector.tensor_scalar(
                    out=qt_sb[:, dt_, ntk * QT:(ntk + 1) * QT], in0=ps[:, :],
                    scalar1=C["bq_col"][:, dt_:dt_ + 1], scalar2=None,
                    op0=ALU.add)

    def _kv_tiles(n0, n1):
        """zT (from LN or gather) + K-proj + V_aug for token tiles n0..n1."""
        if zb_dram is None:
            for n in range(n0, n1):
                zb = _emit_ln_tile(nc, pools, C, x_src_d, n, blk)
                _transpose_tile(nc, pools, C, zb[:, :], zTh[n // HNT],
                                (n % HNT) * P, f"z{blk}_{n}")
        else:
            # one XBAR transpose per gather half, straight from DRAM; on
            # the SP queue its wait-NoOps only delay later residual loads
            assert (n0, n1) in ((0, HNT), (HNT, NTS))
            h = n0 // HNT
            nc.sync.dma_start_transpose(zTh[h][:, :, :], zb_dram[h])
        for dt_ in range(DT):
            for ntk in range(n0 * P // QT, n1 * P // QT):
                zt_src = zTh[ntk * QT // (S // 2)]
                col = ntk * QT % (S // 2)
                ps = psB.tile([P, QT], F32, tag="acc",
                              name=f"pk{blk}_{dt_}_{ntk}")
                for ki in range(DT):
                    nc.tensor.matmul(
                        ps[:, :],
                        lhsT=C["wkT"][:, ki, dt_ * P:(dt_ + 1) * P],
                        rhs=zt_src[:, ki, col:col + QT],
                        start=(ki == 0), stop=(ki == DT - 1))
                nc.vector.tensor_scalar(
                    out=kt_sb[:, dt_, ntk * QT:(ntk + 1) * QT], in0=ps[:, :],
                    scalar1=C["bk_col"][:, dt_:dt_ + 1], scalar2=None,
                    op0=ALU.add)
        for n in range(n0, n1):
            ps = psB.tile([P, QT], F32, tag="acc", name=f"v{blk}_{n}")
            for ki in range(DT):
                nc.tensor.matmul(
                    ps[:, :D],
                    lhsT=zTh[n // HNT][:, ki, (n % HNT) * P:(n % HNT + 1) * P],
                    rhs=C["wvT"][:, ki, :],
                    start=(ki == 0), stop=(ki == DT - 1))
            # ones-augmented layout; ones at j=DK persist from memset
            nc.vector.tensor_tensor(
                out=v_aug[:, n, 0:VROW].rearrange(
                    "p (h j) -> p h j", h=H, j=DK + 1)[:, :, 0:DK],
                in0=ps[:, :D].rearrange("p (h j) -> p h j", h=H, j=DK),
                in1=C["bv_bc"][:, :].rearrange("p (h j) -> p h j", h=H, j=DK),
                op=ALU.add)

    if premade_zb is None:
        # block 1: all LN local; zTh[0] cols 0..OWN are exactly the own
        # queries. 8-tile chunks let K/V projections interleave with the
        # DVE-bound LN stream instead of waiting for all 32 tiles.
        for c in range(0, NTS, 8):
            _kv_tiles(c, c + 8)
        _q_proj(zTh[0])
    else:
        # block 2: emit gather0-dependent K/V first (ready the moment
        # block 1 ends), then the own-LN-dependent Q path, then the
        # gather1-dependent second half.
        _kv_tiles(0, HNT)
        zTq = C["zTq"]
        for n in range(NOT):
            _transpose_tile(nc, pools, C, premade_zb[n][:, :], zTq, n * P,
                            f"q{blk}_{n}")
        _q_proj(zTq)
        _kv_tiles(HNT, NTS)

    # ---- attention (own queries) + per-qt output projection ----
    n_groups = (NKC + KCG - 1) // KCG
    for ntk in range(NQT):
        ot = otp.tile([P, DT, QT], BF16, tag="ot", name=f"ot{blk}_{ntk}")
        for hp in range(DT):
            pv = [psB.tile([P, QT], F32, tag="acc",
                           name=f"pv{blk}_{ntk}_{hp}_{i}") for i in range(2)]
            for g in range(n_groups):
                kcs = list(range(g * KCG, min(NKC, (g + 1) * KCG)))
                w = len(kcs) * QT
                for half in range(2):   # head pair on partitions 0-63/64-127
                    lo = half * DK
                    st = psA.tile([P, KCG * QT], F32, tag="st",
                                  name=f"st{blk}_{ntk}_{hp}_{g}_{half}")
                    for j, kc in enumerate(kcs):
                        nc.tensor.matmul(
                            st[:, j * QT:(j + 1) * QT],
                            lhsT=kt_sb[lo:lo + DK, hp, kc * P:(kc + 1) * P],
                            rhs=qt_sb[lo:lo + DK, hp,
                                      ntk * QT:(ntk + 1) * QT],
                            start=True, stop=True)
                    ste = ste_pool.tile([P, KCG * QT], FP8, tag="ste",
                                        name=f"se{blk}_{ntk}_{hp}_{g}_{half}")
                    nc.scalar.activation(out=ste[:, :w], in_=st[:, :w],
                                         func=AF.Exp, scale=1.0 / 8.0)
                    h = 2 * hp + half
                    # P*V: fp8 DoubleRow over kc pairs (2 key chunks per
                    # matmul at 0.5 cyc/row), odd remainder plain fp8.
                    j = 0
                    while j < len(kcs):
                        kc = kcs[j]
                        if j + 1 < len(kcs):
                            nc.tensor.matmul(
                                pv[half][0:DK + 1, :],
                                lhsT=v_aug[:, kc:kc + 2,
                                           h * (DK + 1):(h + 1) * (DK + 1)],
                                rhs=ste[:, j * QT:(j + 2) * QT].rearrange(
                                    "p (k q) -> p k q", k=2, q=QT),
                                start=(kc == 0), stop=(kc + 1 == NKC - 1),
                                perf_mode=DR, skip_group_check=True)
                            j += 2
                        else:
                            nc.tensor.matmul(
                                pv[half][0:DK + 1, :],
                                lhsT=v_aug[:, kc,
                                           h * (DK + 1):(h + 1) * (DK + 1)],
                                rhs=ste[:, j * QT:(j + 1) * QT],
                                start=(kc == 0), stop=(kc == NKC - 1),
                                skip_group_check=True)
                            j += 1
            for half in range(2):
                lo = half * DK
                r_row = work.tile([1, QT], F32R, tag="r_row",
                                  name=f"rr{blk}_{ntk}_{hp}_{half}")
                with nc.allow_low_precision(
                        reason="f32r broadcast of softmax denom"):
                    nc.vector.reciprocal(
                        out=r_row[:, :], in_=pv[half][DK:DK + 1, :])
                r_bc = psA.tile([P, KCG * QT], F32, tag="st",
                                name=f"rb{blk}_{ntk}_{hp}_{half}")
                nc.tensor.matmul(
                    r_bc[0:DK, 0:QT],
                    lhsT=C["ones"][0:1, 0:DK],
                    rhs=r_row[0:1, :],
                    start=True, stop=True)
                r_sb = work.tile([DK, QT], F32, tag="r_sb",
                                 name=f"rs{blk}_{ntk}_{hp}_{half}")
                nc.vector.tensor_copy(out=r_sb[:, :], in_=r_bc[0:DK, 0:QT])
                nc.vector.tensor_tensor(
                    out=ot[lo:lo + DK, hp, :],
                    in0=pv[half][0:DK, :], in1=r_sb[:, :], op=ALU.mult)
        # output projection + bias + residual for this query tile
        for c4 in range(QT // P):
            tok = ntk * QT + c4 * P
            ps = psB.tile([P, QT], F32, tag="acc",
                          name=f"o{blk}_{ntk}_{c4}")
            for ki in range(DT):
                nc.tensor.matmul(
                    ps[:, :D],
                    lhsT=ot[:, ki, c4 * P:(c4 + 1) * P],
                    rhs=C["woT"][:, ki, :],
                    start=(ki == 0), stop=False)
            nc.tensor.matmul(
                ps[:, :D],
                lhsT=C["ones"][0:1, 0:P],
                rhs=C["bo_row"][0:1, :],
                start=False, stop=True, skip_group_check=True)
            xr = work.tile([P, D], F32, tag="x_res",
                           name=f"xr{blk}_{ntk}_{c4}")
            nc.sync.dma_start(out=xr[:, :], in_=_rows(x_src_d, tok, P))
            xo = work.tile([P, D], F32, tag="x_out",
                           name=f"xo{blk}_{ntk}_{c4}")
            nc.vector.tensor_tensor(
                out=xo[:, :], in0=ps[:, :D], in1=xr[:, :], op=ALU.add)
            nc.sync.dma_start(out=_rows(out_d, tok, P), in_=xo[:, :])
        if after_qt is not None:
            after_qt(ntk)


def _build_program(identity_ln):
    nc = bass.Bass("TRN2", target_bir_lowering=False, debug=False,
                   num_devices=8)

    di = {}
    di["xs"] = nc.dram_tensor("xs", [S, D], F32, kind="ExternalInput")
    for w in ("wqT", "wkT", "wvT", "woT"):
        di[w] = nc.dram_tensor(w, [D, D], BF16, kind="ExternalInput")
    di["bq_col"] = nc.dram_tensor("bq_col", [P, DT], F32, kind="ExternalInput")
    di["bk_col"] = nc.dram_tensor("bk_col", [P, DT], F32, kind="ExternalInput")
    di["bv_bc"] = nc.dram_tensor("bv_bc", [P, D], F32, kind="ExternalInput")
    di["bo_row"] = nc.dram_tensor("bo_row", [1, D], F32R, kind="ExternalInput")
    for w in ("ra0_bc", "rb0_bc", "ra1_bc", "rb1_bc", "a0_bc", "b0_bc"):
        di[w] = nc.dram_tensor(w, [P, D], F32, kind="ExternalInput")
    di["ones_in"] = nc.dram_tensor("ones_in", [1, P], F32R,
                                   kind="ExternalInput")
    di["ident_in"] = nc.dram_tensor("ident_in", [P, P], BF16,
                                    kind="ExternalInput")
    out_d = nc.dram_tensor("out", [OWN, D], F32, kind="ExternalOutput")
    x2_d = nc.dram_tensor("x2buf", [OWN, D], F32)        # internal
    # split gather: half h carries each member's own rows [h*512:(h+1)*512];
    # separate tensors so first-half consumers never falsely depend on the
    # second collective.
    gin_d = [nc.dram_tensor(f"gin{h}", [OWN // 2, D], BF16) for h in range(2)]
    gout_d = [nc.dram_tensor(f"gout{h}", [S // 2, D], BF16) for h in range(2)]

    with tile.TileContext(nc) as tc:
        with tc.tile_pool(name="const", bufs=1) as const, \
             tc.tile_pool(name="work", bufs=3) as work, \
             tc.tile_pool(name="ot", bufs=2) as otp, \
             tc.tile_pool(name="ste", bufs=6) as ste_pool, \
             tc.tile_pool(name="psA", bufs=2, space="PSUM") as psA, \
             tc.tile_pool(name="psB", bufs=2, space="PSUM") as psB:

            pools = {"work": work, "psA": psA, "psB": psB,
                     "ste": ste_pool, "ot": otp}

            C = {}
            for wname in ("wqT", "wkT", "wvT", "woT"):
                C[wname] = const.tile([P, DT, D], BF16, name=wname)
                nc.sync.dma_start(
                    out=C[wname][:, :, :],
                    in_=di[wname][:].rearrange("(d p) e -> p d e", p=P))
            for wname in ("bq_col", "bk_col", "bv_bc"):
                C[wname] = const.tile(list(di[wname].shape), F32, name=wname)
                nc.sync.dma_start(out=C[wname][:], in_=di[wname][:])
            C["bo_row"] = const.tile([1, D], F32R, name="bo_row")
            nc.sync.dma_start(out=C["bo_row"][:], in_=di["bo_row"][:])
            for wname in ("ra0_bc", "rb0_bc", "ra1_bc", "rb1_bc",
                          "a0_bc", "b0_bc"):
                C[wname] = const.tile([P, D], F32, name=wname)
                nc.sync.dma_start(out=C[wname][:, :], in_=di[wname][:])
            C["ones"] = const.tile([1, P], F32R, name="ones")
            nc.sync.dma_start(out=C["ones"][:, :], in_=di["ones_in"][:])
            C["ident"] = const.tile([P, P], BF16, name="ident")
            nc.sync.dma_start(out=C["ident"][:, :], in_=di["ident_in"][:])

            C["zTh"] = [const.tile([P, DT, S // 2], BF16, name=f"zT{h}")
                        for h in range(2)]
            C["zTq"] = const.tile([P, DT, OWN], BF16, name="zTq")
            C["qt"] = const.tile([P, DT, OWN], BF16, name="qt")
            C["kt"] = const.tile([P, DT, S], BF16, name="kt")
            C["v_aug"] = const.tile([P, NTS, VROW2], FP8, name="v_aug")
            nc.vector.memset(C["v_aug"][:, :, :], 1.0)

            C["identity_ln"] = identity_ln
            C1 = dict(C)
            C1["ra_bc"], C1["rb_bc"] = C["ra0_bc"], C["rb0_bc"]
            C2 = dict(C)
            C2["ra_bc"], C2["rb_bc"] = C["ra1_bc"], C["rb1_bc"]

            # block-2 LN of OWN rows interleaves into block-1 attention:
            # as each query tile's x2 rows land, run the pure-DVE LN chain,
            # stage the bf16 result into that half's gather input, and kick
            # the half-AllGather that publishes it group-wide. The first
            # gather runs concurrently with block-1's second query tile.
            zb2 = [None] * NOT

            def _b2_ln(ntk):
                for n in range(ntk * 4, ntk * 4 + 4):
                    zb2[n] = _emit_ln_tile(nc, pools, C2, x2_d[:], n, 1)
                    nc.sync.dma_start(
                        out=_rows(gin_d[ntk][:], (n % 4) * P, P),
                        in_=zb2[n][:, :])
                nc.gpsimd.collective_compute(
                    "AllGather", mybir.AluOpType.bypass,
                    replica_groups=CC_GROUPS,
                    ins=[gin_d[ntk][:].opt()], outs=[gout_d[ntk][:].opt()])

            _build_block(nc, pools, C1, di["xs"][:], x2_d[:], 0,
                         after_qt=_b2_ln)
            _build_block(nc, pools, C2, x2_d[:], out_d[:], 1,
                         zb_dram=[g[:] for g in gout_d], premade_zb=zb2)

    _fix_sync_waits(nc)
    return nc


_NC_CACHE = {}


def _get_nc(identity_ln=True):
    if identity_ln not in _NC_CACHE:
        _NC_CACHE[identity_ln] = _build_program(identity_ln)
    return _NC_CACHE[identity_ln]


def _prep_inputs(x, a0, b0, ra0, rb0, ra1, rb1,
                 wq, bq, wk, bk, wv, bv, wo, bo):
    bf = ml_dtypes.bfloat16
    base = {
        "wqT": np.ascontiguousarray(np.asarray(wq, np.float32).T).astype(bf),
        "wkT": np.ascontiguousarray(np.asarray(wk, np.float32).T).astype(bf),
        "wvT": np.ascontiguousarray(np.asarray(wv, np.float32).T).astype(bf),
        "woT": np.ascontiguousarray(np.asarray(wo, np.float32).T).astype(bf),
        "bq_col": np.ascontiguousarray(
            np.asarray(bq, np.float32).reshape(DT, P).T),
        "bk_col": np.ascontiguousarray(
            np.asarray(bk, np.float32).reshape(DT, P).T),
        "bv_bc": np.ascontiguousarray(
            np.broadcast_to(np.asarray(bv, np.float32), (P, D))),
        "bo_row": np.asarray(bo, np.float32).reshape(1, D).copy(),
        "ra0_bc": np.ascontiguousarray(
            np.broadcast_to(np.asarray(ra0, np.float32), (P, D))),
        "rb0_bc": np.ascontiguousarray(
            np.broadcast_to(np.asarray(rb0, np.float32), (P, D))),
        "ra1_bc": np.ascontiguousarray(
            np.broadcast_to(np.asarray(ra1, np.float32), (P, D))),
        "rb1_bc": np.ascontiguousarray(
            np.broadcast_to(np.asarray(rb1, np.float32), (P, D))),
        "a0_bc": np.ascontiguousarray(
            np.broadcast_to(np.asarray(a0, np.float32), (P, D))),
        "b0_bc": np.ascontiguousarray(
            np.broadcast_to(np.asarray(b0, np.float32), (P, D))),
        "ones_in": np.ones((1, P), np.float32),
        "ident_in": np.eye(P, dtype=np.float32).astype(bf),
    }
    x = np.asarray(x, np.float32)
    in_maps = []
    for c in range(8):
        b, q0 = c // GRP, (c % GRP) * OWN
        m = dict(base)
        # rotate tokens so this core's output shard sits at rows 0..OWN
        m["xs"] = np.ascontiguousarray(
            np.concatenate([x[b, q0:], x[b, :q0]], axis=0))
        in_maps.append(m)
    return in_maps


def kernel(**inputs):
    identity_ln = all(
        bool(np.all(np.asarray(inputs[k], np.float32) == v))
        for k, v in (("a0", 1.0), ("b0", 0.0), ("ra0", 1.0), ("rb0", 0.0),
                     ("ra1", 1.0), ("rb1", 0.0)))
    nc = _get_nc(identity_ln)
    in_maps = _prep_inputs(**inputs)
    res = run_bass_kernel_spmd(nc, in_maps, list(range(8)))
    B = inputs["x"].shape[0]
    out = np.empty((B, S, D), np.float32)
    for c in range(8):
        b, q0 = c // GRP, (c % GRP) * OWN
        out[b, q0:q0 + OWN, :] = res.results[c]["out"]
    return out


if __name__ == "__main__":
    rng = np.random.default_rng(0)
    ins = {
        "x": rng.standard_normal((2, S, D)).astype(np.float32),
        "a0": np.ones(D, np.float32), "b0": np.zeros(D, np.float32),
        "ra0": np.ones(D, np.float32), "rb0": np.zeros(D, np.float32),
        "ra1": np.ones(D, np.float32), "rb1": np.zeros(D, np.float32),
        "wq": (rng.standard_normal((D, D)) * 0.02).astype(np.float32),
        "bq": np.zeros(D, np.float32),
        "wk": (rng.standard_normal((D, D)) * 0.02).astype(np.float32),
        "bk": np.zeros(D, np.float32),
        "wv": (rng.standard_normal((D, D)) * 0.02).astype(np.float32),
        "bv": np.zeros(D, np.float32),
        "wo": (rng.standard_normal((D, D)) * 0.02).astype(np.float32),
        "bo": np.zeros(D, np.float32),
    }
    out = kernel(**ins)
    print("kernel ran, out shape", out.shape, out.dtype)
